# revision 1
# baseline (speedup 1.0000x reference)
"""Trainium2 Bass kernel for nn_MGHD_13486197310275 (gnn_message_passing).

Two SPMD launches on 8 NeuronCores:
  1. Mamba stage, data-parallel over graphs (8 graphs/core), SSD chunked scan.
     Matmuls run as float32r (ap>=256) or bf16 (small tiles) - 4x the fp32
     rate. Also emits per-check-node GNN messages M = relu(proj @ Wmsg + b)
     so launch 2 is pure gather + scatter.
  2. GNN edge stage, node-parallel (9992 dst nodes/core). Check-src edges
     gather precomputed M rows via a few large batched indirect DMAs
     (amortizing the ~1us SWDGE fixed cost); segment-sum via one-hot matmuls
     (one-hot built host-side, DMA'd as bf16). Var-src edges contribute
     count[dst] * relu(b_msg), a rank-1 term folded into the same PSUM
     accumulation.

Host work between launches is index preprocessing / data movement only.
"""
import os
import sys
from contextlib import ExitStack

sys.path.insert(0, "/opt/trn_rl_repo")

import numpy as np

import concourse.bass as bass
import concourse.mybir as mybir
import concourse.tile as tile
from concourse import bacc
from concourse.bass import AP
from concourse.masks import make_identity

F32 = mybir.dt.float32
F32R = mybir.dt.float32r
BF16 = mybir.dt.bfloat16
I32 = mybir.dt.int32
AF = mybir.ActivationFunctionType
ALU = mybir.AluOpType

# ---- problem constants ----
N_IN, N_HID, N_OUT = 16, 128, 2
D_MODEL, D_STATE, D_CONV = 256, 64, 4
D_INNER = 512
NHEADS = 8
HEADDIM = 64
CONV_DIM = 640
D_IN_PROJ = 1160
L = 624
LP = 640
Q = 128
NCH = 5
NPG = 1249
BATCH = 64
N_NODES = BATCH * NPG
NCORE = 8
GPC = 8
NPC = N_NODES // NCORE       # 9992
NWIN = (NPC + 127) // 128    # 79
LG = GPC * L                 # 4992
NCHK = BATCH * L             # 39936 check nodes globally
MROWS = NCHK + 128           # M table rows (last 128 are zero padding)


def bcast_inner(ap, rep):
    """[P, h] -> [P, (h, rep)]: replicate each free element rep times."""
    return AP(ap.tensor, ap.offset, [ap.ap[0], ap.ap[1], [0, rep]])


def bcast_outer(ap, rep):
    """[P, q] -> [P, (rep, q)]: repeat the whole free block rep times."""
    return AP(ap.tensor, ap.offset, [ap.ap[0], [0, rep], ap.ap[1]])


def r(ap):
    """Bitcast fp32 AP to float32r for 4x matmul rate (needs ap_size>=256)."""
    return ap.bitcast(F32R)


_TABLES_PATCHED = False


def _steer_act_tables():
    """Steer the act-table placement pass so Exp and Ln resolve to the
    combined natural_log_exp table (both live there on hw), avoiding a
    1.3us table reload on every Ln<->Exp switch. Set order (and thus the
    act_func_set_id each name maps to) is preserved; only the contents of
    the earlier sets are filtered so the greedy first-match lands on the
    combined set."""
    global _TABLES_PATCHED
    if True:
        return  # disabled: suspected HW act-table id mismatch
    import functools

    import concourse.bacc as bacc_mod
    from concourse.hw_specs import get_activation_tables as _real

    COMBO = "natural_log_exp_and_others"

    @functools.lru_cache(None)
    def steered(arch):
        tabs = _real(arch)
        if COMBO not in tabs:
            return tabs
        out = {}
        before = True
        for name, funcs in tabs.items():
            if name == COMBO:
                before = False
            if before:
                funcs = set(funcs) - {AF.Exp, AF.Ln}
            out[name] = funcs
        return out

    bacc_mod.get_activation_tables = steered
    _TABLES_PATCHED = True


# ======================================================================
# Launch 1: Mamba (+ per-check-node message precompute)
# ======================================================================
def build_mamba(nc, tc, dram):
    io = {}

    def dt_in(name, shape, dtype=F32):
        h = dram.tile(shape, dtype, kind="ExternalInput")
        io[name] = h
        return h

    chkT = dt_in("chkT", [N_IN, LG], BF16)
    Wemb = dt_in("Wemb", [N_IN, D_MODEL])
    b_embT = dt_in("b_embT", [128, 2])
    Win = dt_in("Win", [128, 2 * D_IN_PROJ])
    b_z_row = dt_in("b_z_row", [1, D_INNER])
    diagW = dt_in("diagW", [128, 5 * D_CONV * 128], BF16)
    conv_w5 = dt_in("conv_w5", [128, 5 * D_CONV])
    conv_b5 = dt_in("conv_b5", [128, 5])
    b_xBC5 = dt_in("b_xBC5", [128, 5])
    b_in_dt = dt_in("b_in_dt", [NHEADS, 1])
    dt_bias_in = dt_in("dt_bias_in", [NHEADS, 1])
    A_log_in = dt_in("A_log_in", [NHEADS, 1])
    Dcol_rm = dt_in("Dcol_rm", [1, D_INNER])
    normw_col = dt_in("normw_col", [128, 4])
    Woutp = dt_in("Woutp", [128, 4 * D_MODEL])
    b_outpT = dt_in("b_outpT", [128, 2])
    Wproj = dt_in("Wproj", [128, 2 * N_IN])
    b_projT = dt_in("b_projT", [N_IN, 1])
    Wmsg_aug = dt_in("Wmsg_aug", [N_IN + 1, N_HID])
    projT = dram.tile([N_IN, LG], F32, kind="ExternalOutput")
    io["projT"] = projT
    Mout = dram.tile([GPC * L, N_HID], BF16, kind="ExternalOutput")
    io["Mout"] = Mout

    with ExitStack() as ctx:
        cp = ctx.enter_context(tc.tile_pool(name="const", bufs=1))
        gp = ctx.enter_context(tc.tile_pool(name="gbuf", bufs=1))
        pp = ctx.enter_context(tc.tile_pool(name="ps1", bufs=4, space="PSUM"))
        ppb = ctx.enter_context(tc.tile_pool(name="ps2", bufs=1, space="PSUM"))
        pcp = ctx.enter_context(tc.tile_pool(name="ps3", bufs=2, space="PSUM"))

        ident = cp.tile([128, 128], F32)
        make_identity(nc, ident[:])
        identb = cp.tile([128, 128], BF16)
        nc.vector.tensor_copy(identb[:], ident[:])
        ones_col = cp.tile([1, 128], F32)
        nc.vector.memset(ones_col[:], 1.0)
        eps_col = cp.tile([128, 1], F32)
        nc.vector.memset(eps_col[:], 1e-5)
        one_col = cp.tile([NHEADS, 1], F32)
        nc.vector.memset(one_col[:], 1.0)
        Wemb_sb = cp.tile([N_IN, D_MODEL], F32)
        nc.sync.dma_start(Wemb_sb[:], Wemb[:])
        Wemb_b = cp.tile([N_IN, D_MODEL], BF16)
        nc.vector.tensor_copy(Wemb_b[:], Wemb_sb[:])
        b_embT_sb = cp.tile([128, 2], F32)
        nc.sync.dma_start(b_embT_sb[:], b_embT[:])

        # persistent phase-A outputs
        xBC_all = cp.tile([128, GPC * 5 * LP], BF16)
        zsilu_all = cp.tile([128, GPC * NCH * D_INNER], BF16)

        # =========================== PHASE A ===========================
        actx = ExitStack()
        aw = actx.enter_context(tc.tile_pool(name="wtsA", bufs=1))
        ga = actx.enter_context(tc.tile_pool(name="workA", bufs=1))

        chkT_sb = aw.tile([N_IN, LG], BF16)
        nc.sync.dma_start(chkT_sb[:], chkT[:])
        Win_f = aw.tile([128, 2 * D_IN_PROJ], F32)
        nc.sync.dma_start(Win_f[:], Win[:])
        Win_sb = aw.tile([128, 2 * D_IN_PROJ], BF16)
        nc.vector.tensor_copy(Win_sb[:], Win_f[:])
        b_z_sb = aw.tile([1, D_INNER], F32)
        nc.sync.dma_start(b_z_sb[:], b_z_row[:])
        b_z_b = aw.tile([1, D_INNER], BF16)
        nc.vector.tensor_copy(b_z_b[:], b_z_sb[:])
        ones_b = aw.tile([1, 128], BF16)
        nc.vector.memset(ones_b[:], 1.0)
        diagW_sb = aw.tile([128, 5 * D_CONV * 128], BF16)
        nc.sync.dma_start(diagW_sb[:], diagW[:])
        conv_w5_sb = aw.tile([128, 5 * D_CONV], F32)
        nc.sync.dma_start(conv_w5_sb[:], conv_w5[:])
        conv_b5_sb = aw.tile([128, 5], F32)
        nc.sync.dma_start(conv_b5_sb[:], conv_b5[:])
        b_xBC5_sb = aw.tile([128, 5], F32)
        nc.sync.dma_start(b_xBC5_sb[:], b_xBC5[:])
        conv_bias_sb = aw.tile([128, 5], F32)
        for ct in range(5):
            nc.vector.tensor_reduce(
                conv_bias_sb[:, ct:ct + 1],
                conv_w5_sb[:, ct * D_CONV:(ct + 1) * D_CONV],
                axis=mybir.AxisListType.X, op=ALU.add,
            )
        nc.vector.tensor_tensor(
            conv_bias_sb[:], conv_bias_sb[:], b_xBC5_sb[:], ALU.mult
        )
        nc.vector.tensor_tensor(
            conv_bias_sb[:], conv_bias_sb[:], conv_b5_sb[:], ALU.add
        )

        for g in range(GPC):
            xBC = xBC_all[:, g * 5 * LP:(g + 1) * 5 * LP]
            zsilu = zsilu_all[:, g * NCH * D_INNER:(g + 1) * NCH * D_INNER]

            embT = ga.tile([128, 2 * L], BF16, tag="embT")
            for ch in range(2):
                for ls in range(2):
                    pe = pp.tile([128, 312], F32, tag="ps")
                    nc.tensor.matmul(
                        pe[:],
                        lhsT=Wemb_b[:, ch * 128:(ch + 1) * 128],
                        rhs=chkT_sb[:, g * L + ls * 312: g * L + (ls + 1) * 312],
                        start=True, stop=True,
                    )
                    nc.vector.tensor_scalar_add(
                        embT[:, ch * L + ls * 312: ch * L + (ls + 1) * 312],
                        pe[:], b_embT_sb[:, ch:ch + 1],
                    )

            xBCraw = ga.tile([128, 5 * (4 + LP)], BF16, tag="xBCraw")
            for ct in range(5):
                base = ct * (4 + LP)
                nc.vector.memset(xBCraw[:, base:base + 4], 0.0)
                nc.vector.memset(xBCraw[:, base + 3 + L:base + 4 + LP], 0.0)
                for ls in range(2):
                    px = pp.tile([128, 312], F32, tag="ps")
                    for kh in range(2):
                        nc.tensor.matmul(
                            px[:],
                            lhsT=Win_sb[:, kh * D_IN_PROJ + D_INNER + ct * 128:
                                        kh * D_IN_PROJ + D_INNER + (ct + 1) * 128],
                            rhs=embT[:, kh * L + ls * 312: kh * L + (ls + 1) * 312],
                            start=(kh == 0), stop=(kh == 1),
                        )
                    nc.scalar.activation(
                        xBCraw[:, base + 3 + ls * 312: base + 3 + (ls + 1) * 312],
                        px[:], AF.Copy,
                    )

            nc.vector.memset(zsilu[96:, 4 * D_INNER:], 0.0)
            for tt in range(NCH):
                t0, t1 = tt * 128, min((tt + 1) * 128, L)
                tn = t1 - t0
                pz = pp.tile([128, D_INNER], F32, tag="ps")
                for kh in range(2):
                    nc.tensor.matmul(
                        pz[:tn, :],
                        lhsT=embT[:, kh * L + t0: kh * L + t1],
                        rhs=Win_sb[:, kh * D_IN_PROJ: kh * D_IN_PROJ + D_INNER],
                        start=(kh == 0), stop=False,
                    )
                nc.tensor.matmul(
                    pz[:tn, :], lhsT=ones_b[:, :tn], rhs=b_z_b[:],
                    start=False, stop=True,
                )
                nc.scalar.activation(
                    zsilu[:tn, tt * D_INNER:(tt + 1) * D_INNER], pz[:tn, :], AF.Silu
                )

            for ct in range(5):
                base = ct * (4 + LP)
                for half in range(2):
                    pcv = pp.tile([128, 320], F32, tag="ps")
                    for k in range(D_CONV):
                        nc.tensor.matmul(
                            pcv[:],
                            lhsT=diagW_sb[:, (ct * D_CONV + k) * 128:
                                          (ct * D_CONV + k + 1) * 128],
                            rhs=xBCraw[:, base + k + half * 320:
                                       base + k + half * 320 + 320],
                            start=(k == 0), stop=(k == 3),
                        )
                    nc.scalar.activation(
                        xBC[:, ct * LP + half * 320: ct * LP + (half + 1) * 320],
                        pcv[:], AF.Silu, bias=conv_bias_sb[:, ct:ct + 1],
                    )
                nc.vector.memset(xBC[:, ct * LP + L:(ct + 1) * LP], 0.0)

        actx.close()

        # =========================== PHASE B ===========================
        bw = ctx.enter_context(tc.tile_pool(name="wtsB", bufs=1))
        sp = ctx.enter_context(tc.tile_pool(name="workB", bufs=2))
        sp1 = ctx.enter_context(tc.tile_pool(name="workB1", bufs=1))

        Win_dt_f = bw.tile([128, 2 * NHEADS], F32)
        for kh in range(2):
            nc.sync.dma_start(
                Win_dt_f[:, kh * NHEADS:(kh + 1) * NHEADS],
                Win[:, kh * D_IN_PROJ + D_INNER + CONV_DIM: kh * D_IN_PROJ + D_IN_PROJ],
            )
        Win_dt_sb = bw.tile([128, 2 * NHEADS], BF16)
        nc.vector.tensor_copy(Win_dt_sb[:], Win_dt_f[:])
        b_dt_sb = bw.tile([NHEADS, 1], F32)
        bi_dt_sb = bw.tile([NHEADS, 2], F32)
        nc.sync.dma_start(bi_dt_sb[:, 0:1], b_in_dt[:])
        nc.sync.dma_start(bi_dt_sb[:, 1:2], dt_bias_in[:])
        nc.vector.tensor_tensor(
            b_dt_sb[:], bi_dt_sb[:, 0:1], bi_dt_sb[:, 1:2], ALU.add
        )
        negexpA_sb = bw.tile([NHEADS, 1], F32)
        nc.sync.dma_start(negexpA_sb[:], A_log_in[:])
        nc.scalar.activation(negexpA_sb[:], negexpA_sb[:], AF.Exp)
        nc.vector.tensor_scalar_mul(negexpA_sb[:], negexpA_sb[:], -1.0)
        Dcol_sb = bw.tile([1, D_INNER], F32)
        nc.sync.dma_start(Dcol_sb[:], Dcol_rm[:])
        Dcol_bc = bw.tile([128, D_INNER], F32)
        nc.gpsimd.partition_broadcast(Dcol_bc[:], Dcol_sb[:1, :])
        normw_sb = bw.tile([128, 4], F32)
        nc.sync.dma_start(normw_sb[:], normw_col[:])
        Woutp_sb = bw.tile([128, 4 * D_MODEL], F32)
        nc.sync.dma_start(Woutp_sb[:], Woutp[:])
        for kt in range(4):
            nc.vector.tensor_scalar_mul(
                Woutp_sb[:, kt * D_MODEL:(kt + 1) * D_MODEL],
                Woutp_sb[:, kt * D_MODEL:(kt + 1) * D_MODEL],
                normw_sb[:, kt:kt + 1],
            )
        Woutp_b = bw.tile([128, 4 * D_MODEL], BF16)
        nc.vector.tensor_copy(Woutp_b[:], Woutp_sb[:])
        b_outpT_sb = bw.tile([128, 2], F32)
        nc.sync.dma_start(b_outpT_sb[:], b_outpT[:])
        Wproj_f = bw.tile([128, 2 * N_IN], F32)
        nc.sync.dma_start(Wproj_f[:], Wproj[:])
        Wproj_sb = bw.tile([128, 2 * N_IN], BF16)
        nc.vector.tensor_copy(Wproj_sb[:], Wproj_f[:])
        b_projT_sb = bw.tile([N_IN, 1], F32)
        nc.sync.dma_start(b_projT_sb[:], b_projT[:])
        zeros8 = bw.tile([NHEADS, Q], F32)
        nc.vector.memset(zeros8[:], 0.0)
        Wmsg_sb = bw.tile([N_IN + 1, N_HID], F32)
        nc.sync.dma_start(Wmsg_sb[:], Wmsg_aug[:])
        Wmsg_b = bw.tile([N_IN + 1, N_HID], BF16)
        nc.vector.tensor_copy(Wmsg_b[:], Wmsg_sb[:])

        for g in range(GPC):
            xBC = xBC_all[:, g * 5 * LP:(g + 1) * 5 * LP]
            zsilu = zsilu_all[:, g * NCH * D_INNER:(g + 1) * NCH * D_INNER]

            # recompute embT (dt columns need it; phase-A embT was freed)
            embT_B = gp.tile([128, 2 * L], BF16, tag="embT_B")
            chkT_gB = gp.tile([N_IN, L], BF16, tag="chkT_gB")
            nc.sync.dma_start(chkT_gB[:], chkT[:, g * L:(g + 1) * L])
            for ch in range(2):
                for ls in range(2):
                    pe = pp.tile([128, 312], F32, tag="ps")
                    nc.tensor.matmul(
                        pe[:],
                        lhsT=Wemb_b[:, ch * 128:(ch + 1) * 128],
                        rhs=chkT_gB[:, ls * 312:(ls + 1) * 312],
                        start=True, stop=True,
                    )
                    nc.vector.tensor_scalar_add(
                        embT_B[:, ch * L + ls * 312: ch * L + (ls + 1) * 312],
                        pe[:], b_embT_sb[:, ch:ch + 1],
                    )
            # dt raw + softplus
            dtT = gp.tile([NHEADS, L], F32, tag="dtT")
            dtb = gp.tile([NHEADS, 2 * L], F32, tag="dtb")
            for ls in range(2):
                pdt = pp.tile([NHEADS, 312], F32, tag="ps")
                for kh in range(2):
                    nc.tensor.matmul(
                        pdt[:],
                        lhsT=Win_dt_sb[:, kh * NHEADS:(kh + 1) * NHEADS],
                        rhs=embT_B[:, kh * L + ls * 312: kh * L + (ls + 1) * 312],
                        start=(kh == 0), stop=(kh == 1),
                    )
                nc.vector.tensor_scalar_add(
                    dtb[:, ls * 312:(ls + 1) * 312], pdt[:], b_dt_sb[:, :1]
                )
            nc.scalar.activation(dtb[:, L:], dtb[:, :L], AF.Abs)
            nc.scalar.activation(dtb[:, L:], dtb[:, L:], AF.Exp, scale=-1.0)
            nc.scalar.activation(dtb[:, L:], dtb[:, L:], AF.Ln, bias=one_col[:, :1])
            nc.scalar.activation(dtb[:, :L], dtb[:, :L], AF.Relu)
            nc.vector.tensor_tensor(dtT[:], dtb[:, :L], dtb[:, L:], ALU.add)

            logdA = gp.tile([NHEADS, LP], F32, tag="logdA")
            nc.vector.memset(logdA[:, L:], 0.0)
            nc.vector.tensor_scalar_mul(logdA[:, :L], dtT[:], negexpA_sb[:, :1])
            s_all = gp.tile([NHEADS, LP], F32, tag="s_all")
            for c in range(NCH):
                nc.vector.tensor_tensor_scan(
                    s_all[:, c * Q:(c + 1) * Q],
                    logdA[:, c * Q:(c + 1) * Q], zeros8[:],
                    0.0, ALU.add, ALU.add,
                )
            dtp = gp.tile([NHEADS, LP], F32, tag="dtp")
            nc.vector.memset(dtp[:, L:], 0.0)
            nc.vector.tensor_copy(dtp[:, :L], dtT[:])
            fend = gp.tile([NHEADS, LP], F32, tag="fend")
            for c in range(NCH):
                nc.vector.tensor_scalar(
                    fend[:, c * Q:(c + 1) * Q], s_all[:, c * Q:(c + 1) * Q],
                    s_all[:, (c + 1) * Q - 1:(c + 1) * Q], None, ALU.subtract,
                )
            nc.scalar.activation(fend[:], fend[:], AF.Exp, scale=-1.0)
            nc.vector.tensor_tensor(fend[:], fend[:], dtp[:], ALU.mult)
            expS = gp.tile([NHEADS, LP], F32, tag="expS")
            nc.scalar.activation(expS[:], s_all[:], AF.Exp)

            H = gp.tile([D_STATE, D_INNER], F32, tag="H")
            nc.vector.memset(H[:], 0.0)
            Ct_bf = gp.tile([D_STATE, LP], BF16, tag="Ct_bf")
            nc.sync.dma_start(Ct_bf[:], xBC[D_STATE:2 * D_STATE, 4 * LP:5 * LP])
            Ct_f = gp.tile([D_STATE, LP], F32, tag="Ct_f")
            nc.vector.tensor_copy(Ct_f[:], Ct_bf[:])
            out1T_g = gp.tile([128, 2 * LP], BF16, tag="out1T")

            # batched per-graph DMA: fe_all[0, h*NCH+c] = expS[h, (c+1)*Q-1]
            fe_all = gp.tile([1, NHEADS * NCH], F32, tag="fe_all")
            nc.sync.dma_start(
                fe_all[:1, :].rearrange("p (h c) -> p h c", h=NHEADS),
                expS[:, Q - 1::Q],
            )

            for c in range(NCH):
                t0 = c * Q
                ptr = pp.tile([128, 3 * NHEADS], F32, tag="ps")
                for idx, src2 in enumerate((fend, expS, dtp)):
                    nc.tensor.transpose(
                        ptr[:, idx * NHEADS:(idx + 1) * NHEADS],
                        src2[:, t0:t0 + Q], ident[:NHEADS, :NHEADS],
                    )
                trc = sp.tile([128, 3 * NHEADS], F32, tag="trc")
                nc.vector.tensor_copy(trc[:], ptr[:])
                fendT = trc[:, 0 * NHEADS:1 * NHEADS]
                expST = trc[:, 1 * NHEADS:2 * NHEADS]
                dtpT = trc[:, 2 * NHEADS:3 * NHEADS]

                s_row = sp1.tile([1, NHEADS * Q], F32, tag="s_row")
                nc.sync.dma_start(
                    s_row[:1, :].rearrange("p (h q) -> p h q", h=NHEADS),
                    s_all[:, t0:t0 + Q],
                )
                psI = ppb.tile([128, NHEADS * Q], F32, tag="ps2")
                for hh in range(2):
                    nc.tensor.matmul(
                        psI[:, hh * 512:(hh + 1) * 512],
                        lhsT=ones_col[:],
                        rhs=s_row[:, hh * 512:(hh + 1) * 512],
                        start=True, stop=True,
                    )
                psT = pp.tile([128, NHEADS], F32, tag="ps")
                nc.tensor.transpose(
                    psT[:], s_all[:, t0:t0 + Q], ident[:NHEADS, :NHEADS]
                )
                sT = sp.tile([128, NHEADS], F32, tag="sT")
                nc.vector.tensor_copy(sT[:], psT[:])
                diff = sp1.tile([128, NHEADS * Q], F32, tag="diff")
                nc.vector.tensor_tensor(
                    diff[:], psI[:], bcast_inner(sT[:, :], Q), ALU.subtract
                )
                nc.gpsimd.affine_select(
                    diff[:], diff[:], [[0, NHEADS], [1, Q]], ALU.is_ge,
                    fill=-10000.0, base=0, channel_multiplier=-1,
                )
                expL = sp1.tile([128, NHEADS * Q], F32, tag="expL")
                nc.scalar.activation(expL[:], diff[:], AF.Exp)

                pg = pp.tile([128, Q], F32, tag="ps")
                nc.tensor.matmul(
                    pg[:],
                    lhsT=xBC[:D_STATE, 4 * LP + t0: 4 * LP + t0 + Q],
                    rhs=Ct_bf[:, t0:t0 + Q],
                    start=True, stop=True,
                )
                Gt = sp.tile([128, Q], F32, tag="Gt")
                nc.vector.tensor_copy(Gt[:], pg[:])
                Mt = sp1.tile([128, NHEADS * Q], BF16, tag="Mt")
                nc.gpsimd.tensor_tensor(
                    Mt[:], expL[:], bcast_outer(Gt[:, :], NHEADS), ALU.mult
                )

                pxs = pcp.tile([128, D_INNER], BF16, tag="psc")
                for ct in range(4):
                    nc.tensor.transpose(
                        pxs[:, ct * 128:(ct + 1) * 128],
                        xBC[:, ct * LP + t0: ct * LP + t0 + Q], identb[:],
                    )
                xsT = sp.tile([128, D_INNER], BF16, tag="xsT")
                nc.vector.tensor_copy(xsT[:], pxs[:])
                Xd2 = sp.tile([128, D_INNER], BF16, tag="Xd2")
                nc.vector.tensor_tensor(
                    Xd2[:], xsT[:], bcast_inner(dtpT, HEADDIM), ALU.mult
                )
                Xw = sp.tile([128, D_INNER], BF16, tag="Xw")
                nc.vector.tensor_tensor(
                    Xw[:], xsT[:], bcast_inner(fendT, HEADDIM), ALU.mult
                )

                pbt = pp.tile([128, D_STATE], BF16, tag="ps")
                nc.tensor.transpose(
                    pbt[:], xBC[:D_STATE, 4 * LP + t0: 4 * LP + t0 + Q],
                    identb[:D_STATE, :D_STATE],
                )
                Bt = sp.tile([128, D_STATE], BF16, tag="Bt")
                nc.vector.tensor_copy(Bt[:], pbt[:])

                pY = pcp.tile([128, D_INNER], F32, tag="psc")
                for h in range(NHEADS):
                    nc.tensor.matmul(
                        pY[:, h * HEADDIM:(h + 1) * HEADDIM],
                        lhsT=Mt[:, h * Q:(h + 1) * Q],
                        rhs=Xd2[:, h * HEADDIM:(h + 1) * HEADDIM],
                        start=True, stop=True,
                    )
                pYi = pp.tile([128, D_INNER], F32, tag="ps")
                nc.tensor.matmul(
                    pYi[:], lhsT=Ct_f[:, t0:t0 + Q], rhs=H[:],
                    start=True, stop=True,
                )
                yint = sp.tile([128, D_INNER], F32, tag="yint")
                nc.vector.tensor_tensor(
                    yint[:], pYi[:], bcast_inner(expST, HEADDIM), ALU.mult
                )
                tD = sp.tile([128, D_INNER], F32, tag="tD")
                nc.gpsimd.tensor_tensor(tD[:], xsT[:], Dcol_bc[:], ALU.mult)
                nc.vector.tensor_tensor(tD[:], tD[:], pY[:], ALU.add)
                nc.vector.tensor_tensor(tD[:], tD[:], yint[:], ALU.add)
                yg = sp.tile([128, D_INNER], F32, tag="yg")
                ss = sp.tile([128, 1], F32, tag="ss")
                nc.vector.tensor_tensor(
                    yg[:], tD[:], zsilu[:, c * D_INNER:(c + 1) * D_INNER],
                    ALU.mult,
                )
                nc.scalar.activation(
                    diff[:, :D_INNER], yg[:], AF.Square, accum_out=ss[:]
                )
                lnm = sp.tile([128, 1], F32, tag="lnm")
                nc.scalar.activation(
                    lnm[:], ss[:], AF.Ln, bias=eps_col[:, :1], scale=1.0 / D_INNER
                )
                rinv = sp.tile([128, 1], F32, tag="rinv")
                nc.scalar.activation(rinv[:], lnm[:], AF.Exp, scale=-0.5)
                ygb = sp.tile([128, D_INNER], BF16, tag="ygb")
                nc.vector.tensor_scalar(
                    ygb[:], yg[:], rinv[:, :1], None, ALU.mult,
                )

                # state update (in place): H = H*exp(s_end) + Bt^T @ Xw
                pH = pp.tile([D_STATE, D_INNER], F32, tag="ps")
                nc.tensor.matmul(pH[:], lhsT=Bt[:], rhs=Xw[:],
                                 start=True, stop=True)
                fe_bc = sp.tile([D_STATE, NHEADS], F32, tag="fe_bc")
                nc.gpsimd.partition_broadcast(fe_bc[:], fe_all[:1, c::NCH])
                nc.gpsimd.tensor_tensor(
                    H[:], H[:], bcast_inner(fe_bc[:, :], HEADDIM), ALU.mult
                )
                nc.vector.tensor_tensor(H[:], H[:], pH[:], ALU.add)

                # y transpose + out_proj
                pyt = pcp.tile([128, D_INNER], BF16, tag="psc")
                for kt in range(4):
                    nc.tensor.transpose(
                        pyt[:, kt * 128:(kt + 1) * 128],
                        ygb[:, kt * 128:(kt + 1) * 128], identb[:],
                    )
                ynT = sp.tile([128, D_INNER], BF16, tag="ynT")
                nc.vector.tensor_copy(ynT[:], pyt[:])
                po1 = pp.tile([128, 2 * Q], F32, tag="ps")
                for chalf in range(2):
                    for kt in range(4):
                        nc.tensor.matmul(
                            po1[:, chalf * Q:(chalf + 1) * Q],
                            lhsT=Woutp_b[:, kt * D_MODEL + chalf * 128:
                                         kt * D_MODEL + (chalf + 1) * 128],
                            rhs=ynT[:, kt * 128:(kt + 1) * 128],
                            start=(kt == 0), stop=(kt == 3),
                        )
                for chalf in range(2):
                    nc.vector.tensor_scalar_add(
                        out1T_g[:, chalf * LP + t0: chalf * LP + t0 + Q],
                        po1[:, chalf * Q:(chalf + 1) * Q],
                        b_outpT_sb[:, chalf:chalf + 1],
                    )

            # ---- proj ----
            prT = sp1.tile([N_IN + 1, LP], F32, tag="prT")
            nc.vector.memset(prT[:, :], 1.0)
            for ls in range(2):
                ppj = pp.tile([N_IN, 320], F32, tag="ps")
                for chalf in range(2):
                    nc.tensor.matmul(
                        ppj[:],
                        lhsT=Wproj_sb[:, chalf * N_IN:(chalf + 1) * N_IN],
                        rhs=out1T_g[:, chalf * LP + ls * 320:
                                    chalf * LP + (ls + 1) * 320],
                        start=(chalf == 0), stop=(chalf == 1),
                    )
                nc.vector.tensor_scalar_add(
                    prT[:N_IN, ls * 320:(ls + 1) * 320], ppj[:], b_projT_sb[:, :1]
                )
            nc.sync.dma_start(projT[:, g * L:(g + 1) * L], prT[:N_IN, :L])

            # ---- per-check-node messages: M = relu(proj @ Wmsg + b_msg) ----
            prTb = sp1.tile([N_IN + 1, LP], BF16, tag="prTb")
            nc.vector.tensor_copy(prTb[:, :L], prT[:, :L])
            for tt in range(NCH):
                t0, t1 = tt * 128, min((tt + 1) * 128, L)
                tn = t1 - t0
                pM = pp.tile([128, N_HID], F32, tag="ps")
                nc.tensor.matmul(
                    pM[:tn, :], lhsT=prTb[:, t0:t1], rhs=Wmsg_b[:],
                    start=True, stop=True,
                )
                Msb = sp.tile([128, N_HID], BF16, tag="Msb")
                nc.scalar.activation(Msb[:tn, :], pM[:tn, :], AF.Relu)
                nc.sync.dma_start(
                    Mout[g * L + t0: g * L + t1, :], Msb[:tn, :]
                )

    return io


# ======================================================================
# Launch 2: GNN edge stage (gather M rows + one-hot scatter matmuls)
# ======================================================================
def build_gnn(nc, tc, dram, tmax, nchunk):
    io = {}
    TT = NWIN * tmax                       # total gather tiles
    CHW = (NWIN + nchunk - 1) // nchunk    # windows per gather chunk
    CHT = CHW * tmax                       # tiles per gather chunk

    Mtab = dram.tile([MROWS, N_HID], BF16, kind="ExternalInput")
    io["Mtab"] = Mtab
    srcT = dram.tile([128, TT], I32, kind="ExternalInput")
    io["srcT"] = srcT
    ohT = dram.tile([128, TT * 128], BF16, kind="ExternalInput")
    io["ohT"] = ohT
    cntT = dram.tile([1, NWIN * 128], BF16, kind="ExternalInput")
    io["cntT"] = cntT
    featT_own = dram.tile([N_IN + 1, NWIN * 128], BF16, kind="ExternalInput")
    io["featT_own"] = featT_own
    b_msg_row = dram.tile([1, N_HID], F32, kind="ExternalInput")
    io["b_msg_row"] = b_msg_row
    Wupdf_aug = dram.tile([N_IN + 1, N_HID], F32, kind="ExternalInput")
    io["Wupdf_aug"] = Wupdf_aug
    Wupda = dram.tile([N_HID, N_HID], F32, kind="ExternalInput")
    io["Wupda"] = Wupda
    Wout = dram.tile([N_HID, N_OUT], F32, kind="ExternalInput")
    io["Wout"] = Wout
    b_out_row = dram.tile([1, N_OUT], F32, kind="ExternalInput")
    io["b_out_row"] = b_out_row
    out2 = dram.tile([NWIN * 128, N_OUT], F32, kind="ExternalOutput")
    io["out2"] = out2

    with tile.ExitStack() as ctx:
        cp = ctx.enter_context(tc.tile_pool(name="const", bufs=1))
        mg = ctx.enter_context(tc.tile_pool(name="mgath", bufs=2))
        og = ctx.enter_context(tc.tile_pool(name="ohbuf", bufs=2))
        sp = ctx.enter_context(tc.tile_pool(name="work", bufs=4))
        pp = ctx.enter_context(tc.tile_pool(name="ps1", bufs=4, space="PSUM"))
        pa = ctx.enter_context(tc.tile_pool(name="psagg", bufs=2, space="PSUM"))

        srcT_sb = cp.tile([128, TT], I32)
        nc.sync.dma_start(srcT_sb[:], srcT[:])
        cntT_sb = cp.tile([1, NWIN * 128], BF16)
        nc.sync.dma_start(cntT_sb[:], cntT[:])
        featTo_sb = cp.tile([N_IN + 1, NWIN * 128], BF16)
        nc.sync.dma_start(featTo_sb[:], featT_own[:])
        bmsg_sb = cp.tile([1, N_HID], F32)
        nc.sync.dma_start(bmsg_sb[:], b_msg_row[:])
        rbmsg_b = cp.tile([1, N_HID], BF16)
        nc.scalar.activation(rbmsg_b[:], bmsg_sb[:], AF.Relu)
        Wupdf_sb = cp.tile([N_IN + 1, N_HID], F32)
        nc.sync.dma_start(Wupdf_sb[:], Wupdf_aug[:])
        Wupdf_b = cp.tile([N_IN + 1, N_HID], BF16)
        nc.vector.tensor_copy(Wupdf_b[:], Wupdf_sb[:])
        Wupda_sb = cp.tile([N_HID, N_HID], F32)
        nc.sync.dma_start(Wupda_sb[:], Wupda[:])
        Wupda_b = cp.tile([N_HID, N_HID], BF16)
        nc.vector.tensor_copy(Wupda_b[:], Wupda_sb[:])
        Wout_sb = cp.tile([N_HID, N_OUT], F32)
        nc.sync.dma_start(Wout_sb[:], Wout[:])
        Wout_b = cp.tile([N_HID, N_OUT], BF16)
        nc.vector.tensor_copy(Wout_b[:], Wout_sb[:])
        b_out_sb = cp.tile([1, N_OUT], F32)
        nc.sync.dma_start(b_out_sb[:], b_out_row[:])
        b_out_bc = cp.tile([128, N_OUT], F32)
        nc.gpsimd.partition_broadcast(b_out_bc[:], b_out_sb[:1, :])

        Mg_cur = None
        oh_cur = None
        for w in range(NWIN):
            ci, cw = divmod(w, CHW)
            if cw == 0:
                a = ci * CHT
                b = min((ci + 1) * CHT, TT)
                Mg_cur = mg.tile([128, CHT * N_HID], BF16, tag="Mg")
                for tg in range(b - a):
                    nc.gpsimd.indirect_dma_start(
                        out=Mg_cur[:, tg * N_HID:(tg + 1) * N_HID],
                        out_offset=None,
                        in_=Mtab[:],
                        in_offset=bass.IndirectOffsetOnAxis(
                            ap=srcT_sb[:, a + tg:a + tg + 1], axis=0
                        ),
                    )
                oh_cur = og.tile([128, CHT * 128], BF16, tag="oh")
                nc.sync.dma_start(
                    oh_cur[:, :(b - a) * 128], ohT[:, a * 128: b * 128]
                )

            pagg = pa.tile([N_HID, 128], F32, tag="pagg")
            # rank-1 var-source term: relu(b_msg) x count
            nc.tensor.matmul(
                pagg[:], lhsT=rbmsg_b[:],
                rhs=cntT_sb[:, w * 128:(w + 1) * 128],
                start=True, stop=False,
            )
            for t in range(tmax):
                ti = cw * tmax + t    # tile index within chunk
                nc.tensor.matmul(
                    pagg[:],
                    lhsT=Mg_cur[:, ti * N_HID:(ti + 1) * N_HID],
                    rhs=oh_cur[:, ti * 128:(ti + 1) * 128],
                    start=False, stop=(t == tmax - 1),
                )
            aggT = sp.tile([N_HID, 128], BF16, tag="aggT")
            nc.scalar.activation(aggT[:], pagg[:], AF.Copy)
            ph = pp.tile([N_HID, 128], F32, tag="ps")
            nc.tensor.matmul(
                ph[:], lhsT=Wupdf_b[:],
                rhs=featTo_sb[:, w * 128:(w + 1) * 128],
                start=True, stop=False,
            )
            nc.tensor.matmul(
                ph[:], lhsT=Wupda_b[:], rhs=aggT[:], start=False, stop=True
            )
            h = sp.tile([N_HID, 128], BF16, tag="h")
            nc.scalar.activation(h[:], ph[:], AF.Relu)
            po = pp.tile([128, N_OUT], F32, tag="ps")
            nc.tensor.matmul(po[:], lhsT=h[:], rhs=Wout_b[:], start=True, stop=True)
            ot = sp.tile([128, N_OUT], F32, tag="ot")
            nc.vector.tensor_tensor(ot[:], po[:], b_out_bc[:], ALU.add)
            nc.sync.dma_start(out2[w * 128:(w + 1) * 128, :], ot[:])

    return io


# ======================================================================
# Host driver
# ======================================================================
def _mamba_inputs_per_core(inputs, core):
    chk = inputs["chk"]
    chkT = np.ascontiguousarray(
        chk[core * LG:(core + 1) * LG].T.astype(np.float32)
    )
    conv_w = inputs["conv_w"]
    b_in = inputs["b_in"]
    diagW = np.zeros((5 * D_CONV, 128, 128), np.float32)
    for ct in range(5):
        for k in range(D_CONV):
            np.fill_diagonal(diagW[ct * D_CONV + k], conv_w[ct * 128:(ct + 1) * 128, k])

    def part_major(a, nblk):
        # [nblk*128, C] -> [128, nblk*C]
        c = a.shape[1]
        return a.reshape(nblk, 128, c).transpose(1, 0, 2).reshape(128, nblk * c)

    Wmsg_aug = np.concatenate(
        [inputs["W_msg"].astype(np.float32),
         inputs["b_msg"].astype(np.float32)[None, :]], 0
    )
    d = {
        "chkT": chkT,
        "Wemb": inputs["W_embed"],
        "b_embT": part_major(inputs["b_embed"][:, None], 2),
        "Win": part_major(inputs["W_in"], 2),
        "b_z_row": b_in[None, :D_INNER],
        "diagW": diagW.transpose(1, 0, 2).reshape(128, 5 * D_CONV * 128),
        "conv_w5": part_major(conv_w, 5),
        "conv_b5": part_major(inputs["conv_b"][:, None], 5),
        "b_xBC5": part_major(b_in[D_INNER:D_INNER + CONV_DIM, None], 5),
        "b_in_dt": b_in[D_INNER + CONV_DIM:, None],
        "dt_bias_in": inputs["dt_bias"][:, None],
        "A_log_in": inputs["A_log"][:, None],
        "Dcol_rm": np.repeat(inputs["D_skip"], HEADDIM)[None, :],
        "normw_col": inputs["norm_w"].reshape(4, 128).T,
        "Woutp": part_major(inputs["W_outp"], 4),
        "b_outpT": part_major(inputs["b_outp"][:, None], 2),
        "Wproj": part_major(inputs["W_proj"], 2),
        "b_projT": inputs["b_proj"][:, None],
        "Wmsg_aug": Wmsg_aug,
    }
    import ml_dtypes
    bf16 = ml_dtypes.bfloat16
    out = {k: np.ascontiguousarray(v, np.float32) for k, v in d.items()}
    out["chkT"] = np.ascontiguousarray(chkT.astype(bf16))
    out["diagW"] = np.ascontiguousarray(d["diagW"].astype(bf16))
    return out


LAST_RUN_INFO = {}


def build_l1():
    _steer_act_tables()
    nc1 = bacc.Bacc(None, target_bir_lowering=False)
    with tile.TileContext(nc1) as tc1:
        with tc1.tile_pool(name="dram", bufs=1, space="DRAM") as dram1:
            io1 = build_mamba(nc1, tc1, dram1)
    nc1.compile()
    return nc1, io1


def prep_l1(inputs, io1):
    node_inputs = inputs["node_inputs"].astype(np.float32)
    idx = (np.arange(BATCH)[:, None] * NPG + np.arange(L)[None, :]).reshape(-1)
    chk = node_inputs[idx]
    prep = dict(
        inputs, chk=chk,
        conv_w=inputs["conv_w"].astype(np.float32),
        b_in=inputs["b_in"].astype(np.float32),
    )
    in_maps1 = []
    for c in range(NCORE):
        percore = _mamba_inputs_per_core(prep, c)
        in_maps1.append({io1[k].name: v for k, v in percore.items()})
    return in_maps1, idx


def prep_edges(inputs):
    """Index-only preprocessing: split edges by src type, sort by dst."""
    src = inputs["src_ids"].astype(np.int64)
    dst = inputs["dst_ids"].astype(np.int64)
    is_chk = (src % NPG) < L
    cnt_var = np.bincount(dst[~is_chk], minlength=N_NODES).astype(np.float32)
    s = src[is_chk]
    d = dst[is_chk]
    order = np.argsort(d, kind="stable")
    s, d = s[order], d[order]
    mrow = ((s // NPG) * L + (s % NPG)).astype(np.int32)
    per_core = []
    tmax = 1
    for c in range(NCORE):
        lo, hi = np.searchsorted(d, [c * NPC, (c + 1) * NPC])
        dl = (d[lo:hi] - c * NPC).astype(np.int64)
        mr = mrow[lo:hi]
        win = dl // 128
        cnt = np.bincount(win, minlength=NWIN)
        tmax = max(tmax, int(np.ceil(cnt.max() / 128)))
        per_core.append((dl, mr, cnt))
    return per_core, cnt_var, tmax


def build_l2(tmax, nchunk=4):
    nc2 = bacc.Bacc(None, target_bir_lowering=False)
    with tile.TileContext(nc2) as tc2:
        with tc2.tile_pool(name="dram", bufs=1, space="DRAM") as dram2:
            io2 = build_gnn(nc2, tc2, dram2, tmax, nchunk)
    nc2.compile()
    return nc2, io2


def prep_l2(inputs, io2, M_all, proj, per_core, cnt_var, tmax):
    import ml_dtypes
    bf16 = ml_dtypes.bfloat16

    TT = NWIN * tmax
    W_upd = inputs["W_upd"].astype(np.float32)
    b_upd = inputs["b_upd"].astype(np.float32)
    Wupdf_aug = np.concatenate([W_upd[:N_IN], b_upd[None, :]], 0)
    Wupda = np.ascontiguousarray(W_upd[N_IN:])
    W_out = inputs["W_out"].astype(np.float32)
    b_out = inputs["b_out"].astype(np.float32)
    b_msg = inputs["b_msg"].astype(np.float32)

    Mtab = np.concatenate(
        [np.asarray(M_all, dtype=bf16), np.zeros((128, N_HID), bf16)], 0
    )
    in_maps2 = []
    for c in range(NCORE):
        dl, mr, cnt = per_core[c]
        k_all = len(dl)
        win = dl // 128
        # position of each edge within its window
        starts = np.zeros(NWIN, np.int64)
        starts[1:] = np.cumsum(cnt)[:-1]
        posw = np.arange(k_all) - starts[win]
        tloc = posw // 128
        eloc = posw % 128
        tile_idx = win * tmax + tloc
        src_pad = np.full((TT, 128), NCHK, np.int32)
        src_pad[tile_idx, eloc] = mr
        oh = np.zeros((TT * 128, 128), bf16)
        oh[tile_idx * 128 + eloc, dl % 128] = 1
        srcT_np = np.ascontiguousarray(src_pad.T)
        ohT_np = np.ascontiguousarray(
            oh.reshape(TT, 128, 128).transpose(1, 0, 2).reshape(128, TT * 128)
        )
        cnt_own = np.zeros((1, NWIN * 128), np.float32)
        cnt_own[0, :NPC] = cnt_var[c * NPC:(c + 1) * NPC]
        feat_own = np.zeros((NPC, N_IN), np.float32)
        for gl in range(GPC):
            feat_own[gl * NPG: gl * NPG + L] = \
                proj[(c * GPC + gl) * L:(c * GPC + gl + 1) * L]
        featT_own = np.zeros((N_IN + 1, NWIN * 128), np.float32)
        featT_own[:N_IN, :NPC] = feat_own.T
        featT_own[N_IN] = 1.0
        in_maps2.append({
            io2["Mtab"].name: Mtab,
            io2["srcT"].name: srcT_np,
            io2["ohT"].name: ohT_np,
            io2["cntT"].name: cnt_own.astype(bf16),
            io2["featT_own"].name: featT_own.astype(bf16),
            io2["b_msg_row"].name: b_msg[None, :],
            io2["Wupdf_aug"].name: Wupdf_aug,
            io2["Wupda"].name: Wupda,
            io2["Wout"].name: W_out,
            io2["b_out_row"].name: b_out[None, :],
        })
    return in_maps2


def kernel(**inputs):
    from concourse.bass_utils import run_bass_kernel_spmd

    inputs = {k: np.asarray(v) for k, v in inputs.items()}
    trace = bool(int(os.environ.get("KERNEL_TRACE", "0")))

    nc1, io1 = build_l1()
    in_maps1, idx = prep_l1(inputs, io1)
    res1 = run_bass_kernel_spmd(nc1, in_maps1, core_ids=list(range(NCORE)),
                                trace=trace)
    LAST_RUN_INFO["mamba"] = res1
    proj = np.concatenate(
        [res1.results[c][io1["projT"].name].T for c in range(NCORE)], 0
    )
    M_all = np.concatenate(
        [res1.results[c][io1["Mout"].name] for c in range(NCORE)], 0
    )

    per_core, cnt_var, tmax = prep_edges(inputs)
    nc2, io2 = build_l2(tmax)
    in_maps2 = prep_l2(inputs, io2, M_all, proj, per_core, cnt_var, tmax)
    res2 = run_bass_kernel_spmd(nc2, in_maps2, core_ids=list(range(NCORE)),
                                trace=trace)
    LAST_RUN_INFO["gnn"] = res2
    out = np.concatenate(
        [np.asarray(res2.results[c][io2["out2"].name][:NPC], np.float32)
         for c in range(NCORE)], 0
    )
    return out.astype(np.float32)



# revision 18
# speedup vs baseline: 1.1904x; 1.1904x over previous
"""Trainium2 Bass kernel for nn_MGHD_13486197310275 (gnn_message_passing).

Two SPMD launches on 8 NeuronCores:
  1. Mamba stage, data-parallel over graphs (8 graphs/core), SSD chunked scan.
     Matmuls run as float32r (ap>=256) or bf16 (small tiles) - 4x the fp32
     rate. Also emits per-check-node GNN messages M = relu(proj @ Wmsg + b)
     so launch 2 is pure gather + scatter.
  2. GNN edge stage, node-parallel (9992 dst nodes/core). Check-src edges
     gather precomputed M rows via a few large batched indirect DMAs
     (amortizing the ~1us SWDGE fixed cost); segment-sum via one-hot matmuls
     (one-hot built host-side, DMA'd as bf16). Var-src edges contribute
     count[dst] * relu(b_msg), a rank-1 term folded into the same PSUM
     accumulation.

Host work between launches is index preprocessing / data movement only.
"""
import os
import sys
from contextlib import ExitStack

sys.path.insert(0, "/opt/trn_rl_repo")

import numpy as np

import concourse.bass as bass
import concourse.mybir as mybir
import concourse.tile as tile
from concourse import bacc
from concourse.bass import AP
from concourse.masks import make_identity

F32 = mybir.dt.float32
F32R = mybir.dt.float32r
BF16 = mybir.dt.bfloat16
I32 = mybir.dt.int32
AF = mybir.ActivationFunctionType
ALU = mybir.AluOpType

# ---- problem constants ----
N_IN, N_HID, N_OUT = 16, 128, 2
D_MODEL, D_STATE, D_CONV = 256, 64, 4
D_INNER = 512
NHEADS = 8
HEADDIM = 64
CONV_DIM = 640
D_IN_PROJ = 1160
L = 624
LP = 640
Q = 128
NCH = 5
NPG = 1249
BATCH = 64
N_NODES = BATCH * NPG
NCORE = 8
GPC = 8
NPC = N_NODES // NCORE       # 9992
NWIN = (NPC + 127) // 128    # 79
LG = GPC * L                 # 4992
NCHK = BATCH * L             # 39936 check nodes globally
MROWS = NCHK + 128           # M table rows (last 128 are zero padding)


def bcast_inner(ap, rep):
    """[P, h] -> [P, (h, rep)]: replicate each free element rep times."""
    return AP(ap.tensor, ap.offset, [ap.ap[0], ap.ap[1], [0, rep]])


def bcast_outer(ap, rep):
    """[P, q] -> [P, (rep, q)]: repeat the whole free block rep times."""
    return AP(ap.tensor, ap.offset, [ap.ap[0], [0, rep], ap.ap[1]])


def r(ap):
    """Bitcast fp32 AP to float32r for 4x matmul rate (needs ap_size>=256)."""
    return ap.bitcast(F32R)


_TABLES_PATCHED = False


def _steer_act_tables():
    """Steer the act-table placement pass so Exp and Ln resolve to the
    combined natural_log_exp table (both live there on hw), avoiding a
    1.3us table reload on every Ln<->Exp switch. Set order (and thus the
    act_func_set_id each name maps to) is preserved; only the contents of
    the earlier sets are filtered so the greedy first-match lands on the
    combined set."""
    global _TABLES_PATCHED
    if True:
        return  # disabled: suspected HW act-table id mismatch
    import functools

    import concourse.bacc as bacc_mod
    from concourse.hw_specs import get_activation_tables as _real

    COMBO = "natural_log_exp_and_others"

    @functools.lru_cache(None)
    def steered(arch):
        tabs = _real(arch)
        if COMBO not in tabs:
            return tabs
        out = {}
        before = True
        for name, funcs in tabs.items():
            if name == COMBO:
                before = False
            if before:
                funcs = set(funcs) - {AF.Exp, AF.Ln}
            out[name] = funcs
        return out

    bacc_mod.get_activation_tables = steered
    _TABLES_PATCHED = True


# ======================================================================
# Launch 1: Mamba (+ per-check-node message precompute)
# ======================================================================
def build_mamba(nc, tc, dram):
    io = {}

    def dt_in(name, shape, dtype=F32):
        h = dram.tile(shape, dtype, kind="ExternalInput")
        io[name] = h
        return h

    chkT = dt_in("chkT", [N_IN, LG], BF16)
    Wemb = dt_in("Wemb", [N_IN, D_MODEL])
    b_embT = dt_in("b_embT", [128, 2])
    Win = dt_in("Win", [128, 2 * D_IN_PROJ])
    b_z_row = dt_in("b_z_row", [1, D_INNER])
    diagW = dt_in("diagW", [128, 5 * D_CONV * 128], BF16)
    conv_w5 = dt_in("conv_w5", [128, 5 * D_CONV])
    conv_b5 = dt_in("conv_b5", [128, 5])
    b_xBC5 = dt_in("b_xBC5", [128, 5])
    b_in_dt = dt_in("b_in_dt", [NHEADS, 1])
    dt_bias_in = dt_in("dt_bias_in", [NHEADS, 1])
    A_log_in = dt_in("A_log_in", [NHEADS, 1])
    Dcol_rm = dt_in("Dcol_rm", [1, D_INNER])
    normw_col = dt_in("normw_col", [128, 4])
    Woutp = dt_in("Woutp", [128, 4 * D_MODEL])
    b_outpT = dt_in("b_outpT", [128, 2])
    Wproj = dt_in("Wproj", [128, 2 * N_IN])
    b_projT = dt_in("b_projT", [N_IN, 1])
    Wmsg_aug = dt_in("Wmsg_aug", [N_IN + 1, N_HID])
    projT = dram.tile([N_IN, LG], F32, kind="ExternalOutput")
    io["projT"] = projT
    Mout = dram.tile([GPC * L, N_HID], BF16, kind="ExternalOutput")
    io["Mout"] = Mout

    with ExitStack() as ctx:
        cp = ctx.enter_context(tc.tile_pool(name="const", bufs=1))
        gp = ctx.enter_context(tc.tile_pool(name="gbuf", bufs=1))
        pp = ctx.enter_context(tc.tile_pool(name="ps1", bufs=4, space="PSUM"))
        ppb = ctx.enter_context(tc.tile_pool(name="ps2", bufs=1, space="PSUM"))
        pcp = ctx.enter_context(tc.tile_pool(name="ps3", bufs=2, space="PSUM"))

        ident = cp.tile([128, 128], F32)
        make_identity(nc, ident[:])
        identb = cp.tile([128, 128], BF16)
        nc.vector.tensor_copy(identb[:], ident[:])
        ones_col = cp.tile([1, 128], F32)
        nc.vector.memset(ones_col[:], 1.0)
        eps_col = cp.tile([128, 1], F32)
        nc.vector.memset(eps_col[:], 1e-5)
        one_col = cp.tile([NHEADS, 1], F32)
        nc.vector.memset(one_col[:], 1.0)
        Wemb_sb = cp.tile([N_IN, D_MODEL], F32)
        nc.sync.dma_start(Wemb_sb[:], Wemb[:])
        Wemb_b = cp.tile([N_IN, D_MODEL], BF16)
        nc.vector.tensor_copy(Wemb_b[:], Wemb_sb[:])
        b_embT_sb = cp.tile([128, 2], F32)
        nc.sync.dma_start(b_embT_sb[:], b_embT[:])

        # persistent phase-A outputs
        xBC_all = cp.tile([128, GPC * 5 * LP], BF16)
        zsilu_all = cp.tile([128, GPC * NCH * D_INNER], BF16)

        # =========================== PHASE A ===========================
        actx = ExitStack()
        aw = actx.enter_context(tc.tile_pool(name="wtsA", bufs=1))
        ga = actx.enter_context(tc.tile_pool(name="workA", bufs=1))

        chkT_sb = aw.tile([N_IN, LG], BF16)
        nc.sync.dma_start(chkT_sb[:], chkT[:])
        Win_f = aw.tile([128, 2 * D_IN_PROJ], F32)
        nc.sync.dma_start(Win_f[:], Win[:])
        Win_sb = aw.tile([128, 2 * D_IN_PROJ], BF16)
        nc.vector.tensor_copy(Win_sb[:], Win_f[:])
        b_z_sb = aw.tile([1, D_INNER], F32)
        nc.sync.dma_start(b_z_sb[:], b_z_row[:])
        b_z_b = aw.tile([1, D_INNER], BF16)
        nc.vector.tensor_copy(b_z_b[:], b_z_sb[:])
        ones_b = aw.tile([1, 128], BF16)
        nc.vector.memset(ones_b[:], 1.0)
        diagW_sb = aw.tile([128, 5 * D_CONV * 128], BF16)
        nc.sync.dma_start(diagW_sb[:], diagW[:])
        conv_w5_sb = aw.tile([128, 5 * D_CONV], F32)
        nc.sync.dma_start(conv_w5_sb[:], conv_w5[:])
        conv_b5_sb = aw.tile([128, 5], F32)
        nc.sync.dma_start(conv_b5_sb[:], conv_b5[:])
        b_xBC5_sb = aw.tile([128, 5], F32)
        nc.sync.dma_start(b_xBC5_sb[:], b_xBC5[:])
        conv_bias_sb = aw.tile([128, 5], F32)
        for ct in range(5):
            nc.vector.tensor_reduce(
                conv_bias_sb[:, ct:ct + 1],
                conv_w5_sb[:, ct * D_CONV:(ct + 1) * D_CONV],
                axis=mybir.AxisListType.X, op=ALU.add,
            )
        nc.vector.tensor_tensor(
            conv_bias_sb[:], conv_bias_sb[:], b_xBC5_sb[:], ALU.mult
        )
        nc.vector.tensor_tensor(
            conv_bias_sb[:], conv_bias_sb[:], conv_b5_sb[:], ALU.add
        )

        for g in range(GPC):
            xBC = xBC_all[:, g * 5 * LP:(g + 1) * 5 * LP]
            zsilu = zsilu_all[:, g * NCH * D_INNER:(g + 1) * NCH * D_INNER]

            embT = ga.tile([128, 2 * L], BF16, tag="embT")
            for ch in range(2):
                for ls in range(2):
                    pe = pp.tile([128, 312], F32, tag="ps")
                    nc.tensor.matmul(
                        pe[:],
                        lhsT=Wemb_b[:, ch * 128:(ch + 1) * 128],
                        rhs=chkT_sb[:, g * L + ls * 312: g * L + (ls + 1) * 312],
                        start=True, stop=True,
                    )
                    nc.vector.tensor_scalar_add(
                        embT[:, ch * L + ls * 312: ch * L + (ls + 1) * 312],
                        pe[:], b_embT_sb[:, ch:ch + 1],
                    )

            xBCraw = ga.tile([128, 5 * (4 + LP)], BF16, tag="xBCraw")
            for ct in range(5):
                base = ct * (4 + LP)
                nc.vector.memset(xBCraw[:, base:base + 4], 0.0)
                nc.vector.memset(xBCraw[:, base + 3 + L:base + 4 + LP], 0.0)
                for ls in range(2):
                    px = pp.tile([128, 312], F32, tag="ps")
                    for kh in range(2):
                        nc.tensor.matmul(
                            px[:],
                            lhsT=Win_sb[:, kh * D_IN_PROJ + D_INNER + ct * 128:
                                        kh * D_IN_PROJ + D_INNER + (ct + 1) * 128],
                            rhs=embT[:, kh * L + ls * 312: kh * L + (ls + 1) * 312],
                            start=(kh == 0), stop=(kh == 1),
                        )
                    nc.scalar.activation(
                        xBCraw[:, base + 3 + ls * 312: base + 3 + (ls + 1) * 312],
                        px[:], AF.Copy,
                    )

            nc.vector.memset(zsilu[96:, 4 * D_INNER:], 0.0)
            for tt in range(NCH):
                t0, t1 = tt * 128, min((tt + 1) * 128, L)
                tn = t1 - t0
                pz = pp.tile([128, D_INNER], F32, tag="ps")
                for kh in range(2):
                    nc.tensor.matmul(
                        pz[:tn, :],
                        lhsT=embT[:, kh * L + t0: kh * L + t1],
                        rhs=Win_sb[:, kh * D_IN_PROJ: kh * D_IN_PROJ + D_INNER],
                        start=(kh == 0), stop=False,
                    )
                nc.tensor.matmul(
                    pz[:tn, :], lhsT=ones_b[:, :tn], rhs=b_z_b[:],
                    start=False, stop=True,
                )
                nc.scalar.activation(
                    zsilu[:tn, tt * D_INNER:(tt + 1) * D_INNER], pz[:tn, :], AF.Silu
                )

            for ct in range(5):
                base = ct * (4 + LP)
                for half in range(2):
                    pcv = pp.tile([128, 320], F32, tag="ps")
                    for k in range(D_CONV):
                        nc.tensor.matmul(
                            pcv[:],
                            lhsT=diagW_sb[:, (ct * D_CONV + k) * 128:
                                          (ct * D_CONV + k + 1) * 128],
                            rhs=xBCraw[:, base + k + half * 320:
                                       base + k + half * 320 + 320],
                            start=(k == 0), stop=(k == 3),
                        )
                    nc.scalar.activation(
                        xBC[:, ct * LP + half * 320: ct * LP + (half + 1) * 320],
                        pcv[:], AF.Silu, bias=conv_bias_sb[:, ct:ct + 1],
                    )
                nc.vector.memset(xBC[:, ct * LP + L:(ct + 1) * LP], 0.0)

        actx.close()

        # =========================== PHASE B ===========================
        bw = ctx.enter_context(tc.tile_pool(name="wtsB", bufs=1))
        sp = ctx.enter_context(tc.tile_pool(name="workB", bufs=2))
        sp1 = ctx.enter_context(tc.tile_pool(name="workB1", bufs=1))

        Win_dt_f = bw.tile([128, 2 * NHEADS], F32)
        for kh in range(2):
            nc.sync.dma_start(
                Win_dt_f[:, kh * NHEADS:(kh + 1) * NHEADS],
                Win[:, kh * D_IN_PROJ + D_INNER + CONV_DIM: kh * D_IN_PROJ + D_IN_PROJ],
            )
        Win_dt_sb = bw.tile([128, 2 * NHEADS], BF16)
        nc.vector.tensor_copy(Win_dt_sb[:], Win_dt_f[:])
        b_dt_sb = bw.tile([NHEADS, 1], F32)
        bi_dt_sb = bw.tile([NHEADS, 2], F32)
        nc.sync.dma_start(bi_dt_sb[:, 0:1], b_in_dt[:])
        nc.sync.dma_start(bi_dt_sb[:, 1:2], dt_bias_in[:])
        nc.vector.tensor_tensor(
            b_dt_sb[:], bi_dt_sb[:, 0:1], bi_dt_sb[:, 1:2], ALU.add
        )
        negexpA_sb = bw.tile([NHEADS, 1], F32)
        nc.sync.dma_start(negexpA_sb[:], A_log_in[:])
        nc.scalar.activation(negexpA_sb[:], negexpA_sb[:], AF.Exp)
        nc.vector.tensor_scalar_mul(negexpA_sb[:], negexpA_sb[:], -1.0)
        Dcol_sb = bw.tile([1, D_INNER], F32)
        nc.sync.dma_start(Dcol_sb[:], Dcol_rm[:])
        Dcol_bc = bw.tile([128, D_INNER], F32)
        nc.gpsimd.partition_broadcast(Dcol_bc[:], Dcol_sb[:1, :])
        normw_sb = bw.tile([128, 4], F32)
        nc.sync.dma_start(normw_sb[:], normw_col[:])
        Woutp_sb = bw.tile([128, 4 * D_MODEL], F32)
        nc.sync.dma_start(Woutp_sb[:], Woutp[:])
        for kt in range(4):
            nc.vector.tensor_scalar_mul(
                Woutp_sb[:, kt * D_MODEL:(kt + 1) * D_MODEL],
                Woutp_sb[:, kt * D_MODEL:(kt + 1) * D_MODEL],
                normw_sb[:, kt:kt + 1],
            )
        Woutp_b = bw.tile([128, 4 * D_MODEL], BF16)
        nc.vector.tensor_copy(Woutp_b[:], Woutp_sb[:])
        b_outpT_sb = bw.tile([128, 2], F32)
        nc.sync.dma_start(b_outpT_sb[:], b_outpT[:])
        Wproj_f = bw.tile([128, 2 * N_IN], F32)
        nc.sync.dma_start(Wproj_f[:], Wproj[:])
        Wproj_sb = bw.tile([128, 2 * N_IN], BF16)
        nc.vector.tensor_copy(Wproj_sb[:], Wproj_f[:])
        b_projT_sb = bw.tile([N_IN, 1], F32)
        nc.sync.dma_start(b_projT_sb[:], b_projT[:])
        zeros8 = bw.tile([NHEADS, Q], F32)
        nc.vector.memset(zeros8[:], 0.0)
        Wmsg_sb = bw.tile([N_IN + 1, N_HID], F32)
        nc.sync.dma_start(Wmsg_sb[:], Wmsg_aug[:])
        Wmsg_b = bw.tile([N_IN + 1, N_HID], BF16)
        nc.vector.tensor_copy(Wmsg_b[:], Wmsg_sb[:])

        for g in range(GPC):
            xBC = xBC_all[:, g * 5 * LP:(g + 1) * 5 * LP]
            zsilu = zsilu_all[:, g * NCH * D_INNER:(g + 1) * NCH * D_INNER]

            # recompute embT (dt columns need it; phase-A embT was freed)
            embT_B = gp.tile([128, 2 * L], BF16, tag="embT_B")
            chkT_gB = gp.tile([N_IN, L], BF16, tag="chkT_gB")
            nc.sync.dma_start(chkT_gB[:], chkT[:, g * L:(g + 1) * L])
            for ch in range(2):
                for ls in range(2):
                    pe = pp.tile([128, 312], F32, tag="ps")
                    nc.tensor.matmul(
                        pe[:],
                        lhsT=Wemb_b[:, ch * 128:(ch + 1) * 128],
                        rhs=chkT_gB[:, ls * 312:(ls + 1) * 312],
                        start=True, stop=True,
                    )
                    nc.vector.tensor_scalar_add(
                        embT_B[:, ch * L + ls * 312: ch * L + (ls + 1) * 312],
                        pe[:], b_embT_sb[:, ch:ch + 1],
                    )
            # dt raw + softplus
            dtT = gp.tile([NHEADS, L], F32, tag="dtT")
            dtb = gp.tile([NHEADS, 2 * L], F32, tag="dtb")
            for ls in range(2):
                pdt = pp.tile([NHEADS, 312], F32, tag="ps")
                for kh in range(2):
                    nc.tensor.matmul(
                        pdt[:],
                        lhsT=Win_dt_sb[:, kh * NHEADS:(kh + 1) * NHEADS],
                        rhs=embT_B[:, kh * L + ls * 312: kh * L + (ls + 1) * 312],
                        start=(kh == 0), stop=(kh == 1),
                    )
                nc.vector.tensor_scalar_add(
                    dtb[:, ls * 312:(ls + 1) * 312], pdt[:], b_dt_sb[:, :1]
                )
            nc.scalar.activation(dtb[:, L:], dtb[:, :L], AF.Abs)
            nc.scalar.activation(dtb[:, L:], dtb[:, L:], AF.Exp, scale=-1.0)
            nc.scalar.activation(dtb[:, L:], dtb[:, L:], AF.Ln, bias=one_col[:, :1])
            nc.scalar.activation(dtb[:, :L], dtb[:, :L], AF.Relu)
            nc.vector.tensor_tensor(dtT[:], dtb[:, :L], dtb[:, L:], ALU.add)

            logdA = gp.tile([NHEADS, LP], F32, tag="logdA")
            nc.vector.memset(logdA[:, L:], 0.0)
            nc.vector.tensor_scalar_mul(logdA[:, :L], dtT[:], negexpA_sb[:, :1])
            s_all = gp.tile([NHEADS, LP], F32, tag="s_all")
            for c in range(NCH):
                nc.vector.tensor_tensor_scan(
                    s_all[:, c * Q:(c + 1) * Q],
                    logdA[:, c * Q:(c + 1) * Q], zeros8[:],
                    0.0, ALU.add, ALU.add,
                )
            dtp = gp.tile([NHEADS, LP], F32, tag="dtp")
            nc.vector.memset(dtp[:, L:], 0.0)
            nc.vector.tensor_copy(dtp[:, :L], dtT[:])
            fend = gp.tile([NHEADS, LP], F32, tag="fend")
            for c in range(NCH):
                nc.vector.tensor_scalar(
                    fend[:, c * Q:(c + 1) * Q], s_all[:, c * Q:(c + 1) * Q],
                    s_all[:, (c + 1) * Q - 1:(c + 1) * Q], None, ALU.subtract,
                )
            nc.scalar.activation(fend[:], fend[:], AF.Exp, scale=-1.0)
            nc.vector.tensor_tensor(fend[:], fend[:], dtp[:], ALU.mult)
            expS = gp.tile([NHEADS, LP], F32, tag="expS")
            nc.scalar.activation(expS[:], s_all[:], AF.Exp)

            H = gp.tile([D_STATE, D_INNER], F32, tag="H")
            nc.vector.memset(H[:], 0.0)
            Ct_bf = gp.tile([D_STATE, LP], BF16, tag="Ct_bf")
            nc.sync.dma_start(Ct_bf[:], xBC[D_STATE:2 * D_STATE, 4 * LP:5 * LP])
            Ct_f = gp.tile([D_STATE, LP], F32, tag="Ct_f")
            nc.vector.tensor_copy(Ct_f[:], Ct_bf[:])
            out1T_g = gp.tile([128, 2 * LP], BF16, tag="out1T")

            # batched per-graph DMA: fe_all[0, h*NCH+c] = expS[h, (c+1)*Q-1]
            fe_all = gp.tile([1, NHEADS * NCH], F32, tag="fe_all")
            nc.sync.dma_start(
                fe_all[:1, :].rearrange("p (h c) -> p h c", h=NHEADS),
                expS[:, Q - 1::Q],
            )

            for c in range(NCH):
                t0 = c * Q
                ptr = pp.tile([128, 3 * NHEADS], F32, tag="ps")
                for idx, src2 in enumerate((fend, expS, dtp)):
                    nc.tensor.transpose(
                        ptr[:, idx * NHEADS:(idx + 1) * NHEADS],
                        src2[:, t0:t0 + Q], ident[:NHEADS, :NHEADS],
                    )
                trc = sp.tile([128, 3 * NHEADS], F32, tag="trc")
                nc.vector.tensor_copy(trc[:], ptr[:])
                fendT = trc[:, 0 * NHEADS:1 * NHEADS]
                expST = trc[:, 1 * NHEADS:2 * NHEADS]
                dtpT = trc[:, 2 * NHEADS:3 * NHEADS]

                s_row = sp1.tile([1, NHEADS * Q], F32, tag="s_row")
                nc.sync.dma_start(
                    s_row[:1, :].rearrange("p (h q) -> p h q", h=NHEADS),
                    s_all[:, t0:t0 + Q],
                )
                psI = ppb.tile([128, NHEADS * Q], F32, tag="ps2")
                for hh in range(2):
                    nc.tensor.matmul(
                        psI[:, hh * 512:(hh + 1) * 512],
                        lhsT=ones_col[:],
                        rhs=s_row[:, hh * 512:(hh + 1) * 512],
                        start=True, stop=True,
                    )
                psT = pp.tile([128, NHEADS], F32, tag="ps")
                nc.tensor.transpose(
                    psT[:], s_all[:, t0:t0 + Q], ident[:NHEADS, :NHEADS]
                )
                sT = sp.tile([128, NHEADS], F32, tag="sT")
                nc.vector.tensor_copy(sT[:], psT[:])
                diff = sp1.tile([128, NHEADS * Q], F32, tag="diff")
                nc.vector.tensor_tensor(
                    diff[:], psI[:], bcast_inner(sT[:, :], Q), ALU.subtract
                )
                nc.gpsimd.affine_select(
                    diff[:], diff[:], [[0, NHEADS], [1, Q]], ALU.is_ge,
                    fill=-10000.0, base=0, channel_multiplier=-1,
                )
                expL = sp1.tile([128, NHEADS * Q], F32, tag="expL")
                nc.scalar.activation(expL[:], diff[:], AF.Exp)

                pg = pp.tile([128, Q], F32, tag="ps")
                nc.tensor.matmul(
                    pg[:],
                    lhsT=xBC[:D_STATE, 4 * LP + t0: 4 * LP + t0 + Q],
                    rhs=Ct_bf[:, t0:t0 + Q],
                    start=True, stop=True,
                )
                Gt = sp.tile([128, Q], F32, tag="Gt")
                nc.vector.tensor_copy(Gt[:], pg[:])
                Mt = sp1.tile([128, NHEADS * Q], BF16, tag="Mt")
                nc.gpsimd.tensor_tensor(
                    Mt[:], expL[:], bcast_outer(Gt[:, :], NHEADS), ALU.mult
                )

                pxs = pcp.tile([128, D_INNER], BF16, tag="psc")
                for ct in range(4):
                    nc.tensor.transpose(
                        pxs[:, ct * 128:(ct + 1) * 128],
                        xBC[:, ct * LP + t0: ct * LP + t0 + Q], identb[:],
                    )
                xsT = sp.tile([128, D_INNER], BF16, tag="xsT")
                nc.vector.tensor_copy(xsT[:], pxs[:])
                Xd2 = sp.tile([128, D_INNER], BF16, tag="Xd2")
                nc.vector.tensor_tensor(
                    Xd2[:], xsT[:], bcast_inner(dtpT, HEADDIM), ALU.mult
                )
                Xw = sp.tile([128, D_INNER], BF16, tag="Xw")
                nc.vector.tensor_tensor(
                    Xw[:], xsT[:], bcast_inner(fendT, HEADDIM), ALU.mult
                )

                pbt = pp.tile([128, D_STATE], BF16, tag="ps")
                nc.tensor.transpose(
                    pbt[:], xBC[:D_STATE, 4 * LP + t0: 4 * LP + t0 + Q],
                    identb[:D_STATE, :D_STATE],
                )
                Bt = sp.tile([128, D_STATE], BF16, tag="Bt")
                nc.vector.tensor_copy(Bt[:], pbt[:])

                pY = pcp.tile([128, D_INNER], F32, tag="psc")
                for h in range(NHEADS):
                    nc.tensor.matmul(
                        pY[:, h * HEADDIM:(h + 1) * HEADDIM],
                        lhsT=Mt[:, h * Q:(h + 1) * Q],
                        rhs=Xd2[:, h * HEADDIM:(h + 1) * HEADDIM],
                        start=True, stop=True,
                    )
                pYi = pp.tile([128, D_INNER], F32, tag="ps")
                nc.tensor.matmul(
                    pYi[:], lhsT=Ct_f[:, t0:t0 + Q], rhs=H[:],
                    start=True, stop=True,
                )
                yint = sp.tile([128, D_INNER], F32, tag="yint")
                nc.vector.tensor_tensor(
                    yint[:], pYi[:], bcast_inner(expST, HEADDIM), ALU.mult
                )
                tD = sp.tile([128, D_INNER], F32, tag="tD")
                nc.gpsimd.tensor_tensor(tD[:], xsT[:], Dcol_bc[:], ALU.mult)
                nc.vector.tensor_tensor(tD[:], tD[:], pY[:], ALU.add)
                nc.vector.tensor_tensor(tD[:], tD[:], yint[:], ALU.add)
                yg = sp.tile([128, D_INNER], F32, tag="yg")
                ss = sp.tile([128, 1], F32, tag="ss")
                nc.vector.tensor_tensor(
                    yg[:], tD[:], zsilu[:, c * D_INNER:(c + 1) * D_INNER],
                    ALU.mult,
                )
                nc.scalar.activation(
                    diff[:, :D_INNER], yg[:], AF.Square, accum_out=ss[:]
                )
                lnm = sp.tile([128, 1], F32, tag="lnm")
                nc.scalar.activation(
                    lnm[:], ss[:], AF.Ln, bias=eps_col[:, :1], scale=1.0 / D_INNER
                )
                rinv = sp.tile([128, 1], F32, tag="rinv")
                nc.scalar.activation(rinv[:], lnm[:], AF.Exp, scale=-0.5)
                ygb = sp.tile([128, D_INNER], BF16, tag="ygb")
                nc.vector.tensor_scalar(
                    ygb[:], yg[:], rinv[:, :1], None, ALU.mult,
                )

                # state update (in place): H = H*exp(s_end) + Bt^T @ Xw
                pH = pp.tile([D_STATE, D_INNER], F32, tag="ps")
                nc.tensor.matmul(pH[:], lhsT=Bt[:], rhs=Xw[:],
                                 start=True, stop=True)
                fe_bc = sp.tile([D_STATE, NHEADS], F32, tag="fe_bc")
                nc.gpsimd.partition_broadcast(fe_bc[:], fe_all[:1, c::NCH])
                nc.gpsimd.tensor_tensor(
                    H[:], H[:], bcast_inner(fe_bc[:, :], HEADDIM), ALU.mult
                )
                nc.vector.tensor_tensor(H[:], H[:], pH[:], ALU.add)

                # y transpose + out_proj
                pyt = pcp.tile([128, D_INNER], BF16, tag="psc")
                for kt in range(4):
                    nc.tensor.transpose(
                        pyt[:, kt * 128:(kt + 1) * 128],
                        ygb[:, kt * 128:(kt + 1) * 128], identb[:],
                    )
                ynT = sp.tile([128, D_INNER], BF16, tag="ynT")
                nc.vector.tensor_copy(ynT[:], pyt[:])
                po1 = pp.tile([128, 2 * Q], F32, tag="ps")
                for chalf in range(2):
                    for kt in range(4):
                        nc.tensor.matmul(
                            po1[:, chalf * Q:(chalf + 1) * Q],
                            lhsT=Woutp_b[:, kt * D_MODEL + chalf * 128:
                                         kt * D_MODEL + (chalf + 1) * 128],
                            rhs=ynT[:, kt * 128:(kt + 1) * 128],
                            start=(kt == 0), stop=(kt == 3),
                        )
                for chalf in range(2):
                    nc.vector.tensor_scalar_add(
                        out1T_g[:, chalf * LP + t0: chalf * LP + t0 + Q],
                        po1[:, chalf * Q:(chalf + 1) * Q],
                        b_outpT_sb[:, chalf:chalf + 1],
                    )

            # ---- proj ----
            prT = sp1.tile([N_IN + 1, LP], F32, tag="prT")
            nc.vector.memset(prT[:, :], 1.0)
            for ls in range(2):
                ppj = pp.tile([N_IN, 320], F32, tag="ps")
                for chalf in range(2):
                    nc.tensor.matmul(
                        ppj[:],
                        lhsT=Wproj_sb[:, chalf * N_IN:(chalf + 1) * N_IN],
                        rhs=out1T_g[:, chalf * LP + ls * 320:
                                    chalf * LP + (ls + 1) * 320],
                        start=(chalf == 0), stop=(chalf == 1),
                    )
                nc.vector.tensor_scalar_add(
                    prT[:N_IN, ls * 320:(ls + 1) * 320], ppj[:], b_projT_sb[:, :1]
                )
            nc.sync.dma_start(projT[:, g * L:(g + 1) * L], prT[:N_IN, :L])

            # ---- per-check-node messages: M = relu(proj @ Wmsg + b_msg) ----
            prTb = sp1.tile([N_IN + 1, LP], BF16, tag="prTb")
            nc.vector.tensor_copy(prTb[:, :L], prT[:, :L])
            for tt in range(NCH):
                t0, t1 = tt * 128, min((tt + 1) * 128, L)
                tn = t1 - t0
                pM = pp.tile([128, N_HID], F32, tag="ps")
                nc.tensor.matmul(
                    pM[:tn, :], lhsT=prTb[:, t0:t1], rhs=Wmsg_b[:],
                    start=True, stop=True,
                )
                Msb = sp.tile([128, N_HID], BF16, tag="Msb")
                nc.scalar.activation(Msb[:tn, :], pM[:tn, :], AF.Relu)
                nc.sync.dma_start(
                    Mout[g * L + t0: g * L + t1, :], Msb[:tn, :]
                )

    return io


# ======================================================================
# Launch 1 (v2): Mamba. Quad-wise graph processing, c-outer-g-inner
# chunk interleaving (4 independent chains keep engines fed), packed
# [128,640] per-quad preamble tensors (graph g at partitions g*32..+8),
# ln(dt) folded into the decay matrix, D_skip as a diagonal add to Mt,
# out_proj@proj collapsed host-side to W2 [512,16], RMSNorm deferred to
# per-graph epilogues so the chunk loop only ever uses exp-set
# activations (2 act-table loads per quad instead of 2 per chunk).
# ======================================================================
QUAD = 4


def build_mamba2(nc, tc, dram, has_bz):
    io = {}

    def dt_in(name, shape, dtype=F32):
        h = dram.tile(shape, dtype, kind="ExternalInput")
        io[name] = h
        return h

    chkT = dt_in("chkT", [N_IN, LG], BF16)
    Wemb_b = dt_in("Wemb_b", [N_IN, D_MODEL], BF16)
    b_embT = dt_in("b_embT", [128, 2])
    Wz = dt_in("Wz", [128, 2 * D_INNER], BF16)
    b_z_row = dt_in("b_z_row", [1, D_INNER])
    WxBC = dt_in("WxBC", [128, 2 * CONV_DIM], BF16)
    Wdt = dt_in("Wdt", [128, 2 * NHEADS], BF16)
    b_dt_col = dt_in("b_dt_col", [128, 1])
    negA_col = dt_in("negA_col", [128, 1])
    diagW = dt_in("diagW", [128, 5 * D_CONV * 128], BF16)
    conv_bias5 = dt_in("conv_bias5", [128, 5])
    DdiagC = dt_in("DdiagC", [128, NHEADS * Q], BF16)
    TRI = dt_in("TRI", [128, 128], BF16)
    W2 = dt_in("W2", [128, 4 * N_IN], BF16)
    b2_col = dt_in("b2_col", [N_IN, 1])
    Wmsg_aug = dt_in("Wmsg_aug", [N_IN + 1, N_HID], BF16)
    projT = dram.tile([N_IN, LG], F32, kind="ExternalOutput")
    io["projT"] = projT
    Mout = dram.tile([LG, N_HID], BF16, kind="ExternalOutput")
    io["Mout"] = Mout

    with ExitStack() as ctx:
        cp = ctx.enter_context(tc.tile_pool(name="const", bufs=1))
        qp = ctx.enter_context(tc.tile_pool(name="quad", bufs=1))
        ep = ctx.enter_context(tc.tile_pool(name="embp", bufs=2))
        xrp = ctx.enter_context(tc.tile_pool(name="xraw", bufs=3))
        wp = ctx.enter_context(tc.tile_pool(name="workB", bufs=2))
        wp1 = ctx.enter_context(tc.tile_pool(name="workB1", bufs=2))
        srp = ctx.enter_context(tc.tile_pool(name="srow", bufs=2))
        # PSUM: pa (phase A, 2 banks) | pi (psI, 2 banks, bufs=1)
        # py (1 bank x2) | px (transposes/G, 1 bank x2) | ph (1 bank)
        pa = ctx.enter_context(tc.tile_pool(name="psA", bufs=2, space="PSUM"))
        pi = ctx.enter_context(tc.tile_pool(name="psI", bufs=1, space="PSUM"))
        py = ctx.enter_context(tc.tile_pool(name="psY", bufs=2, space="PSUM"))
        px = ctx.enter_context(tc.tile_pool(name="psX", bufs=2, space="PSUM"))
        phs = ctx.enter_context(tc.tile_pool(name="psH", bufs=1, space="PSUM"))

        ident = cp.tile([128, 128], F32)
        make_identity(nc, ident[:])
        identb = cp.tile([128, 128], BF16)
        nc.vector.tensor_copy(identb[:], ident[:])

        chkT_sb = cp.tile([N_IN, LG], BF16)
        nc.sync.dma_start(chkT_sb[:], chkT[:])
        Wemb_sb = cp.tile([N_IN, D_MODEL], BF16)
        nc.sync.dma_start(Wemb_sb[:], Wemb_b[:])
        b_embT_sb = cp.tile([128, 2], F32)
        nc.sync.dma_start(b_embT_sb[:], b_embT[:])
        Wz_sb = cp.tile([128, 2 * D_INNER], BF16)
        nc.sync.dma_start(Wz_sb[:], Wz[:])
        WxBC_sb = cp.tile([128, 2 * CONV_DIM], BF16)
        nc.sync.dma_start(WxBC_sb[:], WxBC[:])
        Wdt_sb = cp.tile([128, 2 * NHEADS], BF16)
        nc.sync.dma_start(Wdt_sb[:], Wdt[:])
        b_dt_sb = cp.tile([128, 1], F32)
        nc.sync.dma_start(b_dt_sb[:], b_dt_col[:])
        negA_sb = cp.tile([128, 1], F32)
        nc.sync.dma_start(negA_sb[:], negA_col[:])
        diagW_sb = cp.tile([128, 5 * D_CONV * 128], BF16)
        nc.sync.dma_start(diagW_sb[:], diagW[:])
        conv_b5_sb = cp.tile([128, 5], F32)
        nc.sync.dma_start(conv_b5_sb[:], conv_bias5[:])
        Ddiag_sb = cp.tile([128, NHEADS * Q], BF16)
        nc.sync.dma_start(Ddiag_sb[:], DdiagC[:])
        TRI_sb = cp.tile([128, 128], BF16)
        nc.sync.dma_start(TRI_sb[:], TRI[:])
        W2_sb = cp.tile([128, 4 * N_IN], BF16)
        nc.sync.dma_start(W2_sb[:], W2[:])
        b2_sb = cp.tile([N_IN, 1], F32)
        nc.sync.dma_start(b2_sb[:], b2_col[:])
        Wmsg_sb = cp.tile([N_IN + 1, N_HID], BF16)
        nc.sync.dma_start(Wmsg_sb[:], Wmsg_aug[:])
        if has_bz:
            b_z_sb = cp.tile([1, D_INNER], F32)
            nc.sync.dma_start(b_z_sb[:], b_z_row[:])
            b_z_b = cp.tile([1, D_INNER], BF16)
            nc.vector.tensor_copy(b_z_b[:], b_z_sb[:])
            ones_b = cp.tile([1, 128], BF16)
            nc.vector.memset(ones_b[:], 1.0)
        ones_f = cp.tile([1, 128], F32)
        nc.vector.memset(ones_f[:], 1.0)

        # quad-persistent
        xBC_all = cp.tile([128, QUAD * 5 * LP], BF16)
        zsilu_all = cp.tile([128, QUAD * NCH * D_INNER], BF16)
        ygb_all = cp.tile([128, QUAD * NCH * D_INNER], BF16)
        msq = cp.tile([128, QUAD * NCH], F32)
        H_all = cp.tile([D_STATE, QUAD * D_INNER], F32)

        for q in range(2):
            # ================= PHASE A (per graph) =================
            dtq = qp.tile([128, LP], F32, tag="dtq")
            nc.vector.memset(dtq[:, L:], 0.0)
            for gl in range(QUAD):
                g = q * QUAD + gl
                embT = ep.tile([128, 2 * L], BF16, tag="embT")
                for ch in range(2):
                    pe = pa.tile([128, L], F32, tag="psA")
                    nc.tensor.matmul(
                        pe[:], lhsT=Wemb_sb[:, ch * 128:(ch + 1) * 128],
                        rhs=chkT_sb[:, g * L:(g + 1) * L],
                        start=True, stop=True,
                    )
                    nc.vector.tensor_scalar_add(
                        embT[:, ch * L:(ch + 1) * L], pe[:],
                        b_embT_sb[:, ch:ch + 1],
                    )
                # dt raw -> packed dtq rows [gl*32, gl*32+8)
                pdt = pa.tile([NHEADS, L], F32, tag="psdt")
                for kh in range(2):
                    nc.tensor.matmul(
                        pdt[:], lhsT=Wdt_sb[:, kh * NHEADS:(kh + 1) * NHEADS],
                        rhs=embT[:, kh * L:(kh + 1) * L],
                        start=(kh == 0), stop=(kh == 1),
                    )
                nc.vector.tensor_copy(dtq[gl * 32:gl * 32 + NHEADS, :L], pdt[:])
                # z-proj -> silu -> zsilu_all
                zs = zsilu_all[:, gl * NCH * D_INNER:(gl + 1) * NCH * D_INNER]
                for tt in range(NCH):
                    t0, t1 = tt * 128, min((tt + 1) * 128, L)
                    tn = t1 - t0
                    pz = pa.tile([128, D_INNER], F32, tag="psA")
                    for kh in range(2):
                        nc.tensor.matmul(
                            pz[:tn, :], lhsT=embT[:, kh * L + t0:kh * L + t1],
                            rhs=Wz_sb[:, kh * D_INNER:(kh + 1) * D_INNER],
                            start=(kh == 0), stop=(kh == 1) and not has_bz,
                        )
                    if has_bz:
                        nc.tensor.matmul(
                            pz[:tn, :], lhsT=ones_b[:, :tn], rhs=b_z_b[:],
                            start=False, stop=True,
                        )
                    if tn < 128:
                        nc.vector.memset(
                            zs[tn:, tt * D_INNER:(tt + 1) * D_INNER], 0.0)
                    nc.scalar.activation(
                        zs[:tn, tt * D_INNER:(tt + 1) * D_INNER],
                        pz[:tn, :], AF.Silu,
                    )
                # xBC in_proj + conv + silu
                xBC = xBC_all[:, gl * 5 * LP:(gl + 1) * 5 * LP]
                for ct in range(5):
                    pxr = pa.tile([128, L], F32, tag="psA")
                    for kh in range(2):
                        nc.tensor.matmul(
                            pxr[:],
                            lhsT=WxBC_sb[:, kh * CONV_DIM + ct * 128:
                                         kh * CONV_DIM + (ct + 1) * 128],
                            rhs=embT[:, kh * L:(kh + 1) * L],
                            start=(kh == 0), stop=(kh == 1),
                        )
                    xraw = xrp.tile([128, 4 + L + 16], BF16, tag="xraw")
                    nc.vector.memset(xraw[:, :4], 0.0)
                    nc.vector.memset(xraw[:, 4 + L:], 0.0)
                    if ct % 2 == 0:
                        nc.vector.tensor_copy(xraw[:, 4:4 + L], pxr[:])
                    else:
                        nc.scalar.activation(xraw[:, 4:4 + L], pxr[:], AF.Copy)
                    pcv = pa.tile([128, LP], F32, tag="psA")
                    for k in range(D_CONV):
                        nc.tensor.matmul(
                            pcv[:],
                            lhsT=diagW_sb[:, (ct * D_CONV + k) * 128:
                                          (ct * D_CONV + k + 1) * 128],
                            rhs=xraw[:, 1 + k:1 + k + LP],
                            start=(k == 0), stop=(k == 3),
                        )
                    nc.scalar.activation(
                        xBC[:, ct * LP:(ct + 1) * LP], pcv[:], AF.Silu,
                        bias=conv_b5_sb[:, ct:ct + 1],
                    )

            # ================= QUAD PREAMBLE (packed [128, LP]) ======
            nc.vector.tensor_scalar_add(dtq[:, :L], dtq[:, :L], b_dt_sb[:, :1])
            relq = qp.tile([128, LP], F32, tag="relq")
            nc.scalar.activation(relq[:], dtq[:], AF.Relu)
            absq = qp.tile([128, LP], F32, tag="absq")
            nc.scalar.activation(absq[:], dtq[:], AF.Abs)
            nc.scalar.activation(absq[:], absq[:], AF.Exp, scale=-1.0)
            nc.scalar.activation(absq[:], absq[:], AF.Ln, bias=1.0)
            dt_sp = qp.tile([128, LP], F32, tag="dt_sp")
            nc.vector.tensor_tensor(dt_sp[:], relq[:], absq[:], ALU.add)
            lnd = qp.tile([128, LP], F32, tag="lnd")
            nc.scalar.activation(lnd[:, :L], dt_sp[:, :L], AF.Ln)
            nc.vector.memset(lnd[:, L:], 0.0)
            logdA = qp.tile([128, LP], F32, tag="logdA")
            nc.vector.tensor_scalar_mul(logdA[:], dt_sp[:], negA_sb[:, :1])
            s_all = qp.tile([128, LP], F32, tag="s_all")
            zrow = qp.tile([128, Q], F32, tag="zrow")
            nc.vector.memset(zrow[:], 0.0)
            for c in range(NCH):
                nc.vector.tensor_tensor_scan(
                    s_all[:, c * Q:(c + 1) * Q], logdA[:, c * Q:(c + 1) * Q],
                    zrow[:], 0.0, ALU.add, ALU.add,
                )
            tneg = qp.tile([128, LP], F32, tag="tneg")
            nc.vector.tensor_tensor(tneg[:], lnd[:], s_all[:], ALU.subtract)
            fend = qp.tile([128, LP], F32, tag="fend")
            for c in range(NCH):
                nc.vector.tensor_scalar_add(
                    fend[:, c * Q:(c + 1) * Q], tneg[:, c * Q:(c + 1) * Q],
                    s_all[:, (c + 1) * Q - 1:(c + 1) * Q],
                )
            nc.scalar.activation(fend[:], fend[:], AF.Exp)
            expS = qp.tile([128, LP], F32, tag="expS")
            nc.scalar.activation(expS[:], s_all[:], AF.Exp)

            for gl in range(QUAD):
                nc.vector.memset(
                    H_all[:, gl * D_INNER:(gl + 1) * D_INNER], 0.0)

            # ================= PHASE B (c outer, g inner) ============
            for c in range(NCH):
                t0 = c * Q
                # shared transposes for the quad: tneg/fend/expS chunk
                ptr = px.tile([128, 384], F32, tag="ptr")
                for i3, src3 in enumerate((tneg, fend, expS)):
                    nc.tensor.transpose(
                        ptr[:, i3 * 128:(i3 + 1) * 128],
                        src3[:, t0:t0 + Q], ident[:],
                    )
                trc = wp.tile([128, 384], F32, tag="trc")
                nc.vector.tensor_copy(trc[:], ptr[:])

                for gl in range(QUAD):
                    g = q * QUAD + gl
                    xBC = xBC_all[:, gl * 5 * LP:(gl + 1) * 5 * LP]
                    H = H_all[:, gl * D_INNER:(gl + 1) * D_INNER]
                    Bc = xBC[:D_STATE, 4 * LP + t0:4 * LP + t0 + Q]
                    Cc = xBC[D_STATE:2 * D_STATE, 4 * LP + t0:4 * LP + t0 + Q]

                    # s_row DMA: [1, (h, Q)] for this (g, c)
                    s_row = srp.tile([1, NHEADS * Q], F32, tag="s_row")
                    nc.sync.dma_start(
                        s_row[:1, :].rearrange("p (h q) -> p h q", h=NHEADS),
                        s_all[gl * 32:gl * 32 + NHEADS, t0:t0 + Q],
                    )
                    psI = pi.tile([128, NHEADS * Q], F32, tag="psI")
                    for hh in range(2):
                        nc.tensor.matmul(
                            psI[:, hh * 512:(hh + 1) * 512],
                            lhsT=r(ones_f[:]),
                            rhs=r(s_row[:, hh * 512:(hh + 1) * 512]),
                            start=True, stop=False,
                        )
                    nc.tensor.matmul(
                        psI[:], lhsT=identb[:],
                        rhs=bcast_outer(TRI_sb[:, :], NHEADS),
                        start=False, stop=True,
                    )
                    # expL_h = exp(psI_h + tneg_T_h)  (masked by TRI)
                    expL = wp.tile([128, NHEADS * Q], BF16, tag="expL")
                    for h in range(NHEADS):
                        nc.scalar.activation(
                            expL[:, h * Q:(h + 1) * Q],
                            psI[:, h * Q:(h + 1) * Q], AF.Exp,
                            bias=trc[:, gl * 32 + h:gl * 32 + h + 1],
                        )
                    # G = B^T C
                    pg = px.tile([128, Q], F32, tag="pg")
                    nc.tensor.matmul(pg[:], lhsT=Bc, rhs=Cc,
                                     start=True, stop=True)
                    Gt = wp.tile([128, Q], BF16, tag="Gt")
                    nc.vector.tensor_copy(Gt[:], pg[:])
                    # Mt = expL * G + Ddiag
                    Mt = wp.tile([128, NHEADS * Q], BF16, tag="Mt")
                    nc.gpsimd.tensor_tensor(
                        Mt[:], expL[:], bcast_outer(Gt[:, :], NHEADS), ALU.mult)
                    nc.gpsimd.tensor_tensor(
                        Mt[:], Mt[:], Ddiag_sb[:], ALU.add)
                    # xsT transposes
                    pxs = px.tile([128, D_INNER], BF16, tag="pxs")
                    for kt in range(4):
                        nc.tensor.transpose(
                            pxs[:, kt * 128:(kt + 1) * 128],
                            xBC[:, kt * LP + t0:kt * LP + t0 + Q], identb[:],
                        )
                    xsT = wp.tile([128, D_INNER], BF16, tag="xsT")
                    nc.vector.tensor_copy(xsT[:], pxs[:])
                    # pY: intra-chunk per head
                    pY = py.tile([128, D_INNER], F32, tag="pY")
                    for h in range(NHEADS):
                        nc.tensor.matmul(
                            pY[:, h * HEADDIM:(h + 1) * HEADDIM],
                            lhsT=Mt[:, h * Q:(h + 1) * Q],
                            rhs=xsT[:, h * HEADDIM:(h + 1) * HEADDIM],
                            start=True, stop=True,
                        )
                    # pYi: inter-chunk C^T H
                    pYi = py.tile([128, D_INNER], F32, tag="pYi")
                    nc.tensor.matmul(pYi[:], lhsT=r(Cc.bitcast(F32)) if False
                                     else Cc, rhs=H[:],
                                     start=True, stop=True)
                    yint = wp.tile([128, D_INNER], F32, tag="yint")
                    nc.vector.tensor_tensor(
                        yint[:], pYi[:],
                        bcast_inner(trc[:, 256 + gl * 32:256 + gl * 32 + 8],
                                    HEADDIM),
                        ALU.mult,
                    )
                    y1 = wp.tile([128, D_INNER], F32, tag="y1")
                    nc.vector.tensor_tensor(y1[:], pY[:], yint[:], ALU.add)
                    nc.vector.tensor_tensor(
                        ygb_all[:, (gl * NCH + c) * D_INNER:
                                (gl * NCH + c + 1) * D_INNER],
                        y1[:],
                        zsilu_all[:, (gl * NCH + c) * D_INNER:
                                  (gl * NCH + c + 1) * D_INNER],
                        ALU.mult,
                    )
                    nc.scalar.activation(
                        y1[:], ygb_all[:, (gl * NCH + c) * D_INNER:
                                       (gl * NCH + c + 1) * D_INNER],
                        AF.Square,
                        accum_out=msq[:, gl * NCH + c:gl * NCH + c + 1],
                    )
                    # state update: H = H * fe + B^T Xw
                    Xw = wp.tile([128, D_INNER], BF16, tag="Xw")
                    nc.vector.tensor_tensor(
                        Xw[:], xsT[:],
                        bcast_inner(trc[:, 128 + gl * 32:128 + gl * 32 + 8],
                                    HEADDIM),
                        ALU.mult,
                    )
                    pbt = px.tile([128, D_STATE], BF16, tag="pbt")
                    nc.tensor.transpose(
                        pbt[:], Bc, identb[:D_STATE, :D_STATE])
                    Bt = wp.tile([128, D_STATE], BF16, tag="Bt")
                    nc.vector.tensor_copy(Bt[:], pbt[:])
                    fe_bc = wp.tile([D_STATE, NHEADS], F32, tag="fe_bc")
                    nc.gpsimd.partition_broadcast(
                        fe_bc[:],
                        expS[gl * 32:gl * 32 + 1,
                             t0 + Q - 1:t0 + Q].rearrange(
                                 "p q -> p (q q2)", q2=1)
                        if False else
                        s_row[:1, Q - 1::Q],
                    )
                    nc.scalar.activation(
                        fe_bc[:], fe_bc[:], AF.Exp)
                    pH = phs.tile([D_STATE, D_INNER], F32, tag="pH")
                    nc.tensor.matmul(pH[:], lhsT=Bt[:], rhs=Xw[:],
                                     start=True, stop=True)
                    nc.gpsimd.tensor_tensor(
                        H[:], H[:], bcast_inner(fe_bc[:, :], HEADDIM),
                        ALU.mult)
                    nc.vector.tensor_tensor(H[:], H[:], pH[:], ALU.add)

            # ================= EPILOGUE (per graph) ==================
            for gl in range(QUAD):
                g = q * QUAD + gl
                rinv = wp.tile([128, NCH], F32, tag="rinv")
                nc.scalar.activation(
                    rinv[:], msq[:, gl * NCH:(gl + 1) * NCH], AF.Ln,
                    bias=1e-5, scale=1.0 / D_INNER,
                )
                nc.scalar.activation(rinv[:], rinv[:], AF.Exp, scale=-0.5)
                prw = wp1.tile([N_IN + 1, LP], F32, tag="prw")
                nc.vector.memset(prw[N_IN:, :], 1.0)
                for c in range(NCH):
                    ygn = wp.tile([128, D_INNER], BF16, tag="ygn")
                    nc.vector.tensor_scalar_mul(
                        ygn[:], ygb_all[:, (gl * NCH + c) * D_INNER:
                                        (gl * NCH + c + 1) * D_INNER],
                        rinv[:, c:c + 1],
                    )
                    pyt = px.tile([128, D_INNER], BF16, tag="pxs")
                    for kt in range(4):
                        nc.tensor.transpose(
                            pyt[:, kt * 128:(kt + 1) * 128],
                            ygn[:, kt * 128:(kt + 1) * 128], identb[:],
                        )
                    ynT = wp.tile([128, D_INNER], BF16, tag="ynT")
                    nc.vector.tensor_copy(ynT[:], pyt[:])
                    pprw = px.tile([N_IN, Q], F32, tag="pprw")
                    for kt in range(4):
                        nc.tensor.matmul(
                            pprw[:],
                            lhsT=W2_sb[:, kt * N_IN:(kt + 1) * N_IN],
                            rhs=ynT[:, kt * 128:(kt + 1) * 128],
                            start=(kt == 0), stop=(kt == 3),
                        )
                    nc.vector.tensor_scalar_add(
                        prw[:N_IN, t0_col(c):t0_col(c) + Q], pprw[:],
                        b2_sb[:, :1],
                    )
                nc.sync.dma_start(projT[:, g * L:(g + 1) * L], prw[:N_IN, :L])
                prwb = wp1.tile([N_IN + 1, LP], BF16, tag="prwb")
                nc.vector.tensor_copy(prwb[:], prw[:])
                for tt in range(NCH):
                    t0, t1 = tt * 128, min((tt + 1) * 128, L)
                    tn = t1 - t0
                    pM = py.tile([128, N_HID], F32, tag="pY")
                    nc.tensor.matmul(
                        pM[:tn, :], lhsT=prwb[:, t0:t1], rhs=Wmsg_sb[:],
                        start=True, stop=True,
                    )
                    Msb = wp.tile([128, N_HID], BF16, tag="Msb")
                    nc.scalar.activation(Msb[:tn, :], pM[:tn, :], AF.Relu)
                    nc.sync.dma_start(
                        Mout[g * L + t0:g * L + t1, :], Msb[:tn, :])

    return io


def t0_col(c):
    return c * Q


# ======================================================================
# Launch 2 (v2): GNN edge stage. Edge messages are pre-gathered on the
# host (sorted by dst); the device streams them linearly, builds one-hot
# tiles via iota==dl compares, and segment-sums with accumulating
# matmuls. Window-crossing tiles get one matmul per touched window; the
# out-of-window lanes compare to nothing and contribute zero.
# ======================================================================
def build_gnn2(nc, tc, dram, tpw, chunk_t=64):
    """tpw: tiles-per-window (list of NWIN ints, shared across cores).
    Edges are window-packed host-side: window w owns tiles
    [off[w], off[w]+tpw[w]) of eM; one-hot via iota==dlT compare."""
    io = {}
    NW128 = NWIN * 128
    off = np.concatenate([[0], np.cumsum(tpw)]).astype(int)
    ttot = int(off[-1])
    pairs = []  # (tile, win, start, stop)
    for w in range(NWIN):
        for s in range(tpw[w]):
            pairs.append((int(off[w] + s), w, s == 0, s == tpw[w] - 1))
    npair = len(pairs)

    eM = dram.tile([128, ttot * 128], BF16, kind="ExternalInput")
    io["eM"] = eM
    dlT = dram.tile([128, npair], F32, kind="ExternalInput")
    io["dlT"] = dlT
    iotaR = dram.tile([128, 128], F32, kind="ExternalInput")
    io["iotaR"] = iotaR
    featA = dram.tile([18, NW128], BF16, kind="ExternalInput")
    io["featA"] = featA
    WupdA = dram.tile([18, N_HID], F32, kind="ExternalInput")
    io["WupdA"] = WupdA
    Wupda = dram.tile([N_HID, N_HID], F32, kind="ExternalInput")
    io["Wupda"] = Wupda
    Wout = dram.tile([N_HID, N_OUT], F32, kind="ExternalInput")
    io["Wout"] = Wout
    b_out_row = dram.tile([1, N_OUT], F32, kind="ExternalInput")
    io["b_out_row2"] = b_out_row
    out2T = dram.tile([N_OUT, NW128], F32, kind="ExternalOutput")
    io["out2T"] = out2T

    nchunk = (ttot + chunk_t - 1) // chunk_t

    with tile.ExitStack() as ctx:
        cp = ctx.enter_context(tc.tile_pool(name="const", bufs=1))
        mg = ctx.enter_context(tc.tile_pool(name="mgath", bufs=2))
        ohp = ctx.enter_context(tc.tile_pool(name="ohbuf", bufs=8))
        sp = ctx.enter_context(tc.tile_pool(name="work", bufs=4))
        pa = ctx.enter_context(tc.tile_pool(name="psagg", bufs=2, space="PSUM"))
        pb = ctx.enter_context(tc.tile_pool(name="psbig", bufs=2, space="PSUM"))

        dlT_sb = cp.tile([128, npair], F32)
        nc.sync.dma_start(dlT_sb[:], dlT[:])
        iota = cp.tile([128, 128], F32)
        nc.sync.dma_start(iota[:], iotaR[:])
        featA_sb = cp.tile([18, NW128], BF16)
        nc.sync.dma_start(featA_sb[:], featA[:])
        WupdA_sb = cp.tile([18, N_HID], F32)
        nc.sync.dma_start(WupdA_sb[:], WupdA[:])
        WupdA_b = cp.tile([18, N_HID], BF16)
        nc.vector.tensor_copy(WupdA_b[:], WupdA_sb[:])
        Wupda_sb = cp.tile([N_HID, N_HID], F32)
        nc.sync.dma_start(Wupda_sb[:], Wupda[:])
        Wupda_b = cp.tile([N_HID, N_HID], BF16)
        nc.vector.tensor_copy(Wupda_b[:], Wupda_sb[:])
        Wout_sb = cp.tile([N_HID, N_OUT], F32)
        nc.sync.dma_start(Wout_sb[:], Wout[:])
        Wout_b = cp.tile([N_HID, N_OUT], BF16)
        nc.vector.tensor_copy(Wout_b[:], Wout_sb[:])
        b_out_sb = cp.tile([1, N_OUT], F32)
        nc.sync.dma_start(b_out_sb[:], b_out_row[:])

        agg_all = cp.tile([N_HID, NW128], BF16)
        h_all = cp.tile([N_HID, NW128], BF16)

        # --- chunked eM tiles ---
        chunks = [None] * nchunk

        def get_tile(t):
            c = t // chunk_t
            if chunks[c] is None:
                n = min(chunk_t, ttot - c * chunk_t)
                ch = mg.tile([128, chunk_t * 128], BF16, tag="eMchunk")
                nc.sync.dma_start(
                    ch[:, :n * 128],
                    eM[:, c * chunk_t * 128:(c * chunk_t + n) * 128],
                )
                chunks[c] = ch
                if c >= 2:
                    chunks[c - 2] = None  # rotated out (bufs=2)
            off = (t - (t // chunk_t) * chunk_t) * 128
            return chunks[t // chunk_t][:, off:off + 128]

        # --- aggregation: one-hot matmuls per (tile, window) pair ---
        pagg_cur = None
        for pi, (t, w, st, sp_) in enumerate(pairs):
            emt = get_tile(t)
            oh = ohp.tile([128, 128], BF16, tag="oh")
            nc.vector.tensor_scalar(
                oh[:], iota[:], dlT_sb[:, pi:pi + 1], None, ALU.is_equal,
            )
            if st:
                pagg_cur = pa.tile([N_HID, 128], F32, tag="pagg")
            nc.tensor.matmul(pagg_cur[:], lhsT=emt, rhs=oh[:],
                             start=st, stop=sp_)
            if sp_:
                nc.scalar.activation(
                    agg_all[:, w * 128:(w + 1) * 128], pagg_cur[:], AF.Copy,
                )

        # --- update + readout in wide strips ---
        SW = 512
        NSTRIP = (NW128 + SW - 1) // SW
        ones_row = cp.tile([1, SW], F32)
        nc.vector.memset(ones_row[:], 1.0)
        for j in range(NSTRIP):
            c0, c1 = j * SW, min((j + 1) * SW, NW128)
            cw = c1 - c0
            ph = pb.tile([N_HID, SW], F32, tag="ph")
            nc.tensor.matmul(ph[:, :cw], lhsT=WupdA_b[:],
                             rhs=featA_sb[:, c0:c1], start=True, stop=False)
            nc.tensor.matmul(ph[:, :cw], lhsT=Wupda_b[:],
                             rhs=agg_all[:, c0:c1], start=False, stop=True)
            nc.scalar.activation(h_all[:, c0:c1], ph[:, :cw], AF.Relu)
            po = pb.tile([N_OUT, SW], F32, tag="po")
            nc.tensor.matmul(po[:, :cw], lhsT=Wout_b[:], rhs=h_all[:, c0:c1],
                             start=True, stop=False)
            nc.tensor.matmul(po[:, :cw], lhsT=b_out_sb[:],
                             rhs=ones_row[:, :cw], start=False, stop=True)
            ot = sp.tile([N_OUT, SW], F32, tag="ot")
            nc.vector.tensor_copy(ot[:, :cw], po[:, :cw])
            nc.sync.dma_start(out2T[:, c0:c1], ot[:, :cw])

    return io


def build_gnn(nc, tc, dram, tmax, nchunk):
    io = {}
    TT = NWIN * tmax                       # total gather tiles
    CHW = (NWIN + nchunk - 1) // nchunk    # windows per gather chunk
    CHT = CHW * tmax                       # tiles per gather chunk

    Mtab = dram.tile([MROWS, N_HID], BF16, kind="ExternalInput")
    io["Mtab"] = Mtab
    srcT = dram.tile([128, TT], I32, kind="ExternalInput")
    io["srcT"] = srcT
    ohT = dram.tile([128, TT * 128], BF16, kind="ExternalInput")
    io["ohT"] = ohT
    cntT = dram.tile([1, NWIN * 128], BF16, kind="ExternalInput")
    io["cntT"] = cntT
    featT_own = dram.tile([N_IN + 1, NWIN * 128], BF16, kind="ExternalInput")
    io["featT_own"] = featT_own
    b_msg_row = dram.tile([1, N_HID], F32, kind="ExternalInput")
    io["b_msg_row"] = b_msg_row
    Wupdf_aug = dram.tile([N_IN + 1, N_HID], F32, kind="ExternalInput")
    io["Wupdf_aug"] = Wupdf_aug
    Wupda = dram.tile([N_HID, N_HID], F32, kind="ExternalInput")
    io["Wupda"] = Wupda
    Wout = dram.tile([N_HID, N_OUT], F32, kind="ExternalInput")
    io["Wout"] = Wout
    b_out_row = dram.tile([1, N_OUT], F32, kind="ExternalInput")
    io["b_out_row"] = b_out_row
    out2 = dram.tile([NWIN * 128, N_OUT], F32, kind="ExternalOutput")
    io["out2"] = out2

    with tile.ExitStack() as ctx:
        cp = ctx.enter_context(tc.tile_pool(name="const", bufs=1))
        mg = ctx.enter_context(tc.tile_pool(name="mgath", bufs=2))
        og = ctx.enter_context(tc.tile_pool(name="ohbuf", bufs=2))
        sp = ctx.enter_context(tc.tile_pool(name="work", bufs=4))
        pp = ctx.enter_context(tc.tile_pool(name="ps1", bufs=4, space="PSUM"))
        pa = ctx.enter_context(tc.tile_pool(name="psagg", bufs=2, space="PSUM"))

        srcT_sb = cp.tile([128, TT], I32)
        nc.sync.dma_start(srcT_sb[:], srcT[:])
        cntT_sb = cp.tile([1, NWIN * 128], BF16)
        nc.sync.dma_start(cntT_sb[:], cntT[:])
        featTo_sb = cp.tile([N_IN + 1, NWIN * 128], BF16)
        nc.sync.dma_start(featTo_sb[:], featT_own[:])
        bmsg_sb = cp.tile([1, N_HID], F32)
        nc.sync.dma_start(bmsg_sb[:], b_msg_row[:])
        rbmsg_b = cp.tile([1, N_HID], BF16)
        nc.scalar.activation(rbmsg_b[:], bmsg_sb[:], AF.Relu)
        Wupdf_sb = cp.tile([N_IN + 1, N_HID], F32)
        nc.sync.dma_start(Wupdf_sb[:], Wupdf_aug[:])
        Wupdf_b = cp.tile([N_IN + 1, N_HID], BF16)
        nc.vector.tensor_copy(Wupdf_b[:], Wupdf_sb[:])
        Wupda_sb = cp.tile([N_HID, N_HID], F32)
        nc.sync.dma_start(Wupda_sb[:], Wupda[:])
        Wupda_b = cp.tile([N_HID, N_HID], BF16)
        nc.vector.tensor_copy(Wupda_b[:], Wupda_sb[:])
        Wout_sb = cp.tile([N_HID, N_OUT], F32)
        nc.sync.dma_start(Wout_sb[:], Wout[:])
        Wout_b = cp.tile([N_HID, N_OUT], BF16)
        nc.vector.tensor_copy(Wout_b[:], Wout_sb[:])
        b_out_sb = cp.tile([1, N_OUT], F32)
        nc.sync.dma_start(b_out_sb[:], b_out_row[:])
        b_out_bc = cp.tile([128, N_OUT], F32)
        nc.gpsimd.partition_broadcast(b_out_bc[:], b_out_sb[:1, :])

        Mg_cur = None
        oh_cur = None
        for w in range(NWIN):
            ci, cw = divmod(w, CHW)
            if cw == 0:
                a = ci * CHT
                b = min((ci + 1) * CHT, TT)
                Mg_cur = mg.tile([128, CHT * N_HID], BF16, tag="Mg")
                for tg in range(b - a):
                    nc.gpsimd.indirect_dma_start(
                        out=Mg_cur[:, tg * N_HID:(tg + 1) * N_HID],
                        out_offset=None,
                        in_=Mtab[:],
                        in_offset=bass.IndirectOffsetOnAxis(
                            ap=srcT_sb[:, a + tg:a + tg + 1], axis=0
                        ),
                    )
                oh_cur = og.tile([128, CHT * 128], BF16, tag="oh")
                nc.sync.dma_start(
                    oh_cur[:, :(b - a) * 128], ohT[:, a * 128: b * 128]
                )

            pagg = pa.tile([N_HID, 128], F32, tag="pagg")
            # rank-1 var-source term: relu(b_msg) x count
            nc.tensor.matmul(
                pagg[:], lhsT=rbmsg_b[:],
                rhs=cntT_sb[:, w * 128:(w + 1) * 128],
                start=True, stop=False,
            )
            for t in range(tmax):
                ti = cw * tmax + t    # tile index within chunk
                nc.tensor.matmul(
                    pagg[:],
                    lhsT=Mg_cur[:, ti * N_HID:(ti + 1) * N_HID],
                    rhs=oh_cur[:, ti * 128:(ti + 1) * 128],
                    start=False, stop=(t == tmax - 1),
                )
            aggT = sp.tile([N_HID, 128], BF16, tag="aggT")
            nc.scalar.activation(aggT[:], pagg[:], AF.Copy)
            ph = pp.tile([N_HID, 128], F32, tag="ps")
            nc.tensor.matmul(
                ph[:], lhsT=Wupdf_b[:],
                rhs=featTo_sb[:, w * 128:(w + 1) * 128],
                start=True, stop=False,
            )
            nc.tensor.matmul(
                ph[:], lhsT=Wupda_b[:], rhs=aggT[:], start=False, stop=True
            )
            h = sp.tile([N_HID, 128], BF16, tag="h")
            nc.scalar.activation(h[:], ph[:], AF.Relu)
            po = pp.tile([128, N_OUT], F32, tag="ps")
            nc.tensor.matmul(po[:], lhsT=h[:], rhs=Wout_b[:], start=True, stop=True)
            ot = sp.tile([128, N_OUT], F32, tag="ot")
            nc.vector.tensor_tensor(ot[:], po[:], b_out_bc[:], ALU.add)
            nc.sync.dma_start(out2[w * 128:(w + 1) * 128, :], ot[:])

    return io


# ======================================================================
# Host driver
# ======================================================================
def _mamba_inputs_per_core(inputs, core):
    chk = inputs["chk"]
    chkT = np.ascontiguousarray(
        chk[core * LG:(core + 1) * LG].T.astype(np.float32)
    )
    conv_w = inputs["conv_w"]
    b_in = inputs["b_in"]
    diagW = np.zeros((5 * D_CONV, 128, 128), np.float32)
    for ct in range(5):
        for k in range(D_CONV):
            np.fill_diagonal(diagW[ct * D_CONV + k], conv_w[ct * 128:(ct + 1) * 128, k])

    def part_major(a, nblk):
        # [nblk*128, C] -> [128, nblk*C]
        c = a.shape[1]
        return a.reshape(nblk, 128, c).transpose(1, 0, 2).reshape(128, nblk * c)

    Wmsg_aug = np.concatenate(
        [inputs["W_msg"].astype(np.float32),
         inputs["b_msg"].astype(np.float32)[None, :]], 0
    )
    d = {
        "chkT": chkT,
        "Wemb": inputs["W_embed"],
        "b_embT": part_major(inputs["b_embed"][:, None], 2),
        "Win": part_major(inputs["W_in"], 2),
        "b_z_row": b_in[None, :D_INNER],
        "diagW": diagW.transpose(1, 0, 2).reshape(128, 5 * D_CONV * 128),
        "conv_w5": part_major(conv_w, 5),
        "conv_b5": part_major(inputs["conv_b"][:, None], 5),
        "b_xBC5": part_major(b_in[D_INNER:D_INNER + CONV_DIM, None], 5),
        "b_in_dt": b_in[D_INNER + CONV_DIM:, None],
        "dt_bias_in": inputs["dt_bias"][:, None],
        "A_log_in": inputs["A_log"][:, None],
        "Dcol_rm": np.repeat(inputs["D_skip"], HEADDIM)[None, :],
        "normw_col": inputs["norm_w"].reshape(4, 128).T,
        "Woutp": part_major(inputs["W_outp"], 4),
        "b_outpT": part_major(inputs["b_outp"][:, None], 2),
        "Wproj": part_major(inputs["W_proj"], 2),
        "b_projT": inputs["b_proj"][:, None],
        "Wmsg_aug": Wmsg_aug,
    }
    import ml_dtypes
    bf16 = ml_dtypes.bfloat16
    out = {k: np.ascontiguousarray(v, np.float32) for k, v in d.items()}
    out["chkT"] = np.ascontiguousarray(chkT.astype(bf16))
    out["diagW"] = np.ascontiguousarray(d["diagW"].astype(bf16))
    return out


LAST_RUN_INFO = {}


def build_l1():
    _steer_act_tables()
    nc1 = bacc.Bacc(None, target_bir_lowering=False)
    with tile.TileContext(nc1) as tc1:
        with tc1.tile_pool(name="dram", bufs=1, space="DRAM") as dram1:
            io1 = build_mamba(nc1, tc1, dram1)
    nc1.compile()
    return nc1, io1


def prep_l1(inputs, io1):
    node_inputs = inputs["node_inputs"].astype(np.float32)
    idx = (np.arange(BATCH)[:, None] * NPG + np.arange(L)[None, :]).reshape(-1)
    chk = node_inputs[idx]
    prep = dict(
        inputs, chk=chk,
        conv_w=inputs["conv_w"].astype(np.float32),
        b_in=inputs["b_in"].astype(np.float32),
    )
    in_maps1 = []
    for c in range(NCORE):
        percore = _mamba_inputs_per_core(prep, c)
        in_maps1.append({io1[k].name: v for k, v in percore.items()})
    return in_maps1, idx


def prep_edges(inputs):
    """Index-only preprocessing: split edges by src type, sort by dst."""
    src = inputs["src_ids"].astype(np.int64)
    dst = inputs["dst_ids"].astype(np.int64)
    is_chk = (src % NPG) < L
    cnt_var = np.bincount(dst[~is_chk], minlength=N_NODES).astype(np.float32)
    s = src[is_chk]
    d = dst[is_chk]
    order = np.argsort(d, kind="stable")
    s, d = s[order], d[order]
    mrow = ((s // NPG) * L + (s % NPG)).astype(np.int32)
    per_core = []
    tmax = 1
    for c in range(NCORE):
        lo, hi = np.searchsorted(d, [c * NPC, (c + 1) * NPC])
        dl = (d[lo:hi] - c * NPC).astype(np.int64)
        mr = mrow[lo:hi]
        win = dl // 128
        cnt = np.bincount(win, minlength=NWIN)
        tmax = max(tmax, int(np.ceil(cnt.max() / 128)))
        per_core.append((dl, mr, cnt))
    return per_core, cnt_var, tmax


def prep_edges2(inputs):
    """Sort check-src edges by dst; window-pack with per-window tile
    counts shared (maxed) across cores so one NEFF serves all cores."""
    src = inputs["src_ids"].astype(np.int64)
    dst = inputs["dst_ids"].astype(np.int64)
    is_chk = (src % NPG) < L
    cnt_var = np.bincount(dst[~is_chk], minlength=N_NODES).astype(np.float32)
    s = src[is_chk]
    d = dst[is_chk]
    order = np.argsort(d, kind="stable")
    s, d = s[order], d[order]
    mrow = ((s // NPG) * L + (s % NPG)).astype(np.int64)
    core_data = []
    counts = np.zeros((NCORE, NWIN), np.int64)
    for c in range(NCORE):
        lo, hi = np.searchsorted(d, [c * NPC, (c + 1) * NPC])
        dl = (d[lo:hi] - c * NPC).astype(np.int64)
        mr = mrow[lo:hi]
        counts[c] = np.bincount(dl // 128, minlength=NWIN)
        core_data.append((dl, mr))
    tpw = np.maximum(1, (counts.max(axis=0) + 127) // 128).astype(int)
    off = np.concatenate([[0], np.cumsum(tpw)]).astype(int)
    ttot = int(off[-1])
    per_core = []
    for c in range(NCORE):
        dl, mr = core_data[c]
        dlw = np.full(ttot * 128, -999.0, np.float32)
        mr_pad = np.zeros(ttot * 128, np.int64)
        estart = np.concatenate([[0], np.cumsum(counts[c])]).astype(int)
        for w in range(NWIN):
            e0, e1 = estart[w], estart[w + 1]
            base = off[w] * 128
            dlw[base:base + (e1 - e0)] = dl[e0:e1] - w * 128
            mr_pad[base:base + (e1 - e0)] = mr[e0:e1]
        dlT = np.ascontiguousarray(
            dlw.reshape(ttot, 128).T)               # [128, ttot]
        per_core.append(dict(mr_pad=mr_pad, dlT=dlT))
    return per_core, cnt_var, tpw, ttot


def prep_l2_v2(inputs, io2, M_all, proj, pc, cnt_var, ttot, core):
    import ml_dtypes
    bf16 = ml_dtypes.bfloat16
    c = core
    NW128 = NWIN * 128

    W_upd = inputs["W_upd"].astype(np.float32)
    b_upd = inputs["b_upd"].astype(np.float32)
    W_out = inputs["W_out"].astype(np.float32)
    b_out = inputs["b_out"].astype(np.float32)
    b_msg = inputs["b_msg"].astype(np.float32)
    Wupda = np.ascontiguousarray(W_upd[N_IN:])
    # var-src edges contribute count * relu(b_msg) to agg; fold through
    # Wupda into an extra featA row.
    v = np.maximum(b_msg, 0.0) @ Wupda          # [N_HID]
    WupdA = np.concatenate(
        [W_upd[:N_IN], b_upd[None, :], v[None, :]], 0
    )  # [18, N_HID]

    Mtab = np.asarray(M_all, dtype=bf16)        # [NCHK, N_HID]
    mr = pc["mr_pad"]
    eM = Mtab[mr]                               # [ttot*128, N_HID]
    eM = np.ascontiguousarray(
        eM.reshape(ttot, 128, N_HID).transpose(1, 0, 2).reshape(128, ttot * N_HID)
    )

    featA = np.zeros((18, NW128), np.float32)
    for gl in range(GPC):
        featA[:N_IN, gl * NPG: gl * NPG + L] = \
            proj[(c * GPC + gl) * L:(c * GPC + gl + 1) * L].T
    featA[N_IN, :NPC] = 1.0
    featA[N_IN + 1, :NPC] = cnt_var[c * NPC:(c + 1) * NPC]

    return {
        io2["eM"].name: eM,
        io2["dlT"].name: pc["dlT"],
        io2["iotaR"].name: np.tile(np.arange(128, dtype=np.float32), (128, 1)),
        io2["featA"].name: featA.astype(bf16),
        io2["WupdA"].name: WupdA,
        io2["Wupda"].name: Wupda,
        io2["Wout"].name: W_out,
        io2["b_out_row2"].name: b_out[None, :],
    }


def build_l2(tmax, nchunk=4):
    nc2 = bacc.Bacc(None, target_bir_lowering=False)
    with tile.TileContext(nc2) as tc2:
        with tc2.tile_pool(name="dram", bufs=1, space="DRAM") as dram2:
            io2 = build_gnn(nc2, tc2, dram2, tmax, nchunk)
    nc2.compile()
    return nc2, io2


def build_l2_v2(tpw):
    nc2 = bacc.Bacc(None, target_bir_lowering=False)
    with tile.TileContext(nc2) as tc2:
        with tc2.tile_pool(name="dram", bufs=1, space="DRAM") as dram2:
            io2 = build_gnn2(nc2, tc2, dram2, tpw)
    nc2.compile()
    return nc2, io2


def prep_l2(inputs, io2, M_all, proj, per_core, cnt_var, tmax):
    import ml_dtypes
    bf16 = ml_dtypes.bfloat16

    TT = NWIN * tmax
    W_upd = inputs["W_upd"].astype(np.float32)
    b_upd = inputs["b_upd"].astype(np.float32)
    Wupdf_aug = np.concatenate([W_upd[:N_IN], b_upd[None, :]], 0)
    Wupda = np.ascontiguousarray(W_upd[N_IN:])
    W_out = inputs["W_out"].astype(np.float32)
    b_out = inputs["b_out"].astype(np.float32)
    b_msg = inputs["b_msg"].astype(np.float32)

    Mtab = np.concatenate(
        [np.asarray(M_all, dtype=bf16), np.zeros((128, N_HID), bf16)], 0
    )
    in_maps2 = []
    for c in range(NCORE):
        dl, mr, cnt = per_core[c]
        k_all = len(dl)
        win = dl // 128
        # position of each edge within its window
        starts = np.zeros(NWIN, np.int64)
        starts[1:] = np.cumsum(cnt)[:-1]
        posw = np.arange(k_all) - starts[win]
        tloc = posw // 128
        eloc = posw % 128
        tile_idx = win * tmax + tloc
        src_pad = np.full((TT, 128), NCHK, np.int32)
        src_pad[tile_idx, eloc] = mr
        oh = np.zeros((TT * 128, 128), bf16)
        oh[tile_idx * 128 + eloc, dl % 128] = 1
        srcT_np = np.ascontiguousarray(src_pad.T)
        ohT_np = np.ascontiguousarray(
            oh.reshape(TT, 128, 128).transpose(1, 0, 2).reshape(128, TT * 128)
        )
        cnt_own = np.zeros((1, NWIN * 128), np.float32)
        cnt_own[0, :NPC] = cnt_var[c * NPC:(c + 1) * NPC]
        feat_own = np.zeros((NPC, N_IN), np.float32)
        for gl in range(GPC):
            feat_own[gl * NPG: gl * NPG + L] = \
                proj[(c * GPC + gl) * L:(c * GPC + gl + 1) * L]
        featT_own = np.zeros((N_IN + 1, NWIN * 128), np.float32)
        featT_own[:N_IN, :NPC] = feat_own.T
        featT_own[N_IN] = 1.0
        in_maps2.append({
            io2["Mtab"].name: Mtab,
            io2["srcT"].name: srcT_np,
            io2["ohT"].name: ohT_np,
            io2["cntT"].name: cnt_own.astype(bf16),
            io2["featT_own"].name: featT_own.astype(bf16),
            io2["b_msg_row"].name: b_msg[None, :],
            io2["Wupdf_aug"].name: Wupdf_aug,
            io2["Wupda"].name: Wupda,
            io2["Wout"].name: W_out,
            io2["b_out_row"].name: b_out[None, :],
        })
    return in_maps2


def kernel(**inputs):
    from concourse.bass_utils import run_bass_kernel_spmd

    inputs = {k: np.asarray(v) for k, v in inputs.items()}
    trace = bool(int(os.environ.get("KERNEL_TRACE", "0")))

    nc1, io1 = build_l1()
    in_maps1, idx = prep_l1(inputs, io1)
    res1 = run_bass_kernel_spmd(nc1, in_maps1, core_ids=list(range(NCORE)),
                                trace=trace)
    LAST_RUN_INFO["mamba"] = res1
    proj = np.concatenate(
        [res1.results[c][io1["projT"].name].T for c in range(NCORE)], 0
    )
    M_all = np.concatenate(
        [res1.results[c][io1["Mout"].name] for c in range(NCORE)], 0
    )

    per_core, cnt_var, tpw, ttot = prep_edges2(inputs)
    nc2, io2 = build_l2_v2(tpw)
    in_maps2 = [
        prep_l2_v2(inputs, io2, M_all, proj, per_core[c], cnt_var, ttot, c)
        for c in range(NCORE)
    ]
    res2 = run_bass_kernel_spmd(nc2, in_maps2, core_ids=list(range(NCORE)),
                                trace=trace)
    LAST_RUN_INFO["gnn"] = res2
    out = np.concatenate(
        [np.asarray(res2.results[c][io2["out2T"].name][:, :NPC], np.float32).T
         for c in range(NCORE)], 0
    )
    return out.astype(np.float32)

